# revision 5
# baseline (speedup 1.0000x reference)
"""Trainium2 Bass kernel for nn_Block_17033840296551 (GNN message passing block).

Data-parallel over batch: 16 images -> 8 cores x 2 images. Each core runs the
full block (g1 conv -> kNN top-8+self -> EdgeConv max-agg -> g2 -> FFN ->
bottleneck -> final BN) on its 2 images with no cross-core communication.

v2 (overlap rewrite):
  * Per-image tiles (distinct tags) so the Tile scheduler software-pipelines
    the two images: image B's matmul/DVE phases fill image A's gather phase
    (gpsimd descriptor-gen is the serial bottleneck at ~18us per 2k-idx
    gather).
  * Self-neighbor removed from the gather (8 idxs/node instead of 9); the
    self term is folded in via qT computed directly in ch-major layout by
    matmul.
  * All residual adds (g2, f2, b3, outer) ride the PSUM accumulation as
    identity matmuls; final BN is fused into the last PSUM evacuation
    (scale/bias). No f32 intermediates in SBUF; x arrives as f16 from host.
  * Norms: n2 broadcast to 128 partitions via all-ones matmul, rsqrt via
    reciprocal_approx_fast + Sqrt activation (the old [1,N] reciprocal was
    6.5us of single-lane DVE).
  * Index wrap buffer built with 12 DMAs (was 39).
"""

import os
import numpy as np

# problem constants (hardcoded per harness contract)
B, C, H, W = 16, 256, 32, 32
N = H * W           # 1024 pixels per image
K = 9
EPS = 1e-5
IMGS_PER_CORE = 2
N_CORES = 8
NEG_BIG = -30000.0

_cache = {}


# --------------------------------------------------------------------------
# host-side preprocessing
# --------------------------------------------------------------------------
def _bn_fold(p):
    g, b, m, v = np.asarray(p, np.float32)
    s = g / np.sqrt(v + EPS)
    t = b - m * s
    return s, t


def _pack_kxm(w_t, part=128):
    """[K, M] -> [part, K//part, M] (partition-major K tiling)."""
    Kd, M = w_t.shape
    kt = Kd // part
    return np.ascontiguousarray(w_t.reshape(kt, part, M).transpose(1, 0, 2))


def _pack_bias(b, part=128):
    n = b.shape[0]
    t = n // part
    return np.ascontiguousarray(b.reshape(t, part).T)  # [part, t]


def _prep_weights(inp):
    f16 = np.float16
    s1, t1 = _bn_fold(inp['g1_bn'])
    Wg1 = s1[:, None] * inp['g1_w']
    s2, t2 = _bn_fold(inp['g2_bn'])
    Wg2 = s2[:, None] * inp['g2_w']
    sf1, tf1 = _bn_fold(inp['f1_bn'])
    Wf1 = sf1[:, None] * inp['f1_w']
    bf1 = sf1 * inp['f1_b'] + tf1
    sf2, tf2 = _bn_fold(inp['f2_bn'])
    Wf2 = sf2[:, None] * inp['f2_w']
    bf2 = sf2 * inp['f2_b'] + tf2
    sb1, tb1 = _bn_fold(inp['b1_bn'])
    Wb1 = sb1[:, None] * inp['b1_w']
    sb2, tb2 = _bn_fold(inp['b2_bn'])
    Wb2 = sb2[:, None, None, None] * inp['b2_w']
    sb3, tb3 = _bn_fold(inp['b3_bn'])
    Wb3 = sb3[:, None] * inp['b3_w']
    sf, tf = _bn_fold(inp['bnf'])
    # final BN absorbs b3's bias: out = sf*(P + tb3) + tf, P = b3conv+h2+x
    btf2 = sf * tb3 + tf

    A = inp['edge_w'][:, :C]
    Bm = inp['edge_w'][:, C:]
    Wp = A - Bm
    Wq = Bm
    bp = inp['edge_b']

    wb2_t = np.zeros((64, 9, 64), f16)
    for dy in range(3):
        for dx in range(3):
            wb2_t[:, dy * 3 + dx, :] = Wb2[:, :, dy, dx].T.astype(f16)

    return {
        'wg1': _pack_kxm(Wg1.T.astype(f16)),           # [128,2,256]
        'wp': _pack_kxm(Wp.T.astype(f16)),             # [128,2,512]
        'wq': _pack_kxm(Wq.T.astype(f16)),             # [128,2,512]
        'wg2': _pack_kxm(Wg2.T.astype(f16)),           # [128,4,256]
        'wf1': _pack_kxm(Wf1.T.astype(f16)),           # [128,2,1024]
        'wf2': _pack_kxm(Wf2.T.astype(f16)),           # [128,8,256]
        'wb1': _pack_kxm(Wb1.T.astype(f16)),           # [128,2,64]
        'wb2': wb2_t,                                   # [64,9,64]
        'wb3': Wb3.T.astype(f16),                       # [64,256]
        'bt1': _pack_bias(t1),                          # [128,2] f32
        'bt2': _pack_bias(t2),
        'bbp': _pack_bias(bp),                          # [128,4]
        'bbf1': _pack_bias(bf1),                        # [128,8]
        'bbf2': _pack_bias(bf2),
        'btb1': np.ascontiguousarray(tb1[:, None].astype(np.float32)),  # [64,1]
        'btb2': np.ascontiguousarray(tb2[:, None].astype(np.float32)),
        'bsf': _pack_bias(sf),
        'btf2': _pack_bias(btf2),
    }


# --------------------------------------------------------------------------
# device kernel builder
# --------------------------------------------------------------------------
def _build_bass():
    import concourse.bass as bass
    import concourse.mybir as mybir
    from concourse import bacc
    from concourse.tile import TileContext
    from concourse.masks import make_identity

    dt = mybir.dt
    F16 = dt.float16
    F32 = dt.float32
    AF = mybir.ActivationFunctionType

    nc = bacc.Bacc()

    # ---- DRAM parameters ----
    xh_d = nc.declare_dram_parameter("xh", [IMGS_PER_CORE, C, N], F16, isOutput=False)
    wg1_d = nc.declare_dram_parameter("wg1", [128, 2, 256], F16, isOutput=False)
    wp_d = nc.declare_dram_parameter("wp", [128, 2, 512], F16, isOutput=False)
    wq_d = nc.declare_dram_parameter("wq", [128, 2, 512], F16, isOutput=False)
    wg2_d = nc.declare_dram_parameter("wg2", [128, 4, 256], F16, isOutput=False)
    wf1_d = nc.declare_dram_parameter("wf1", [128, 2, 1024], F16, isOutput=False)
    wf2_d = nc.declare_dram_parameter("wf2", [128, 8, 256], F16, isOutput=False)
    wb1_d = nc.declare_dram_parameter("wb1", [128, 2, 64], F16, isOutput=False)
    wb2_d = nc.declare_dram_parameter("wb2", [64, 9, 64], F16, isOutput=False)
    wb3_d = nc.declare_dram_parameter("wb3", [64, 256], F16, isOutput=False)
    bt1_d = nc.declare_dram_parameter("bt1", [128, 2], F32, isOutput=False)
    bt2_d = nc.declare_dram_parameter("bt2", [128, 2], F32, isOutput=False)
    bbp_d = nc.declare_dram_parameter("bbp", [128, 4], F32, isOutput=False)
    bbf1_d = nc.declare_dram_parameter("bbf1", [128, 8], F32, isOutput=False)
    bbf2_d = nc.declare_dram_parameter("bbf2", [128, 2], F32, isOutput=False)
    btb1_d = nc.declare_dram_parameter("btb1", [64, 1], F32, isOutput=False)
    btb2_d = nc.declare_dram_parameter("btb2", [64, 1], F32, isOutput=False)
    bsf_d = nc.declare_dram_parameter("bsf", [128, 2], F32, isOutput=False)
    btf2_d = nc.declare_dram_parameter("btf2", [128, 2], F32, isOutput=False)
    q_drams = [nc.dram_tensor(f"q_dram{i}", [N, 512], F16)
               for i in range(IMGS_PER_CORE)]
    out_d = nc.declare_dram_parameter("out", [IMGS_PER_CORE, C, N], F32, isOutput=True)

    sbuf_gather = bool(os.environ.get("KBENCH_SBUFGATHER"))

    with TileContext(nc) as tc:
        import contextlib
        ctx = contextlib.ExitStack()
        with ctx:
            consts = ctx.enter_context(tc.tile_pool(name="consts", bufs=1))
            # per-image working pool: every tag is a dedicated buffer
            pools = [ctx.enter_context(tc.tile_pool(name=f"im{i}", bufs=1))
                     for i in range(IMGS_PER_CORE)]
            pool_gath = ctx.enter_context(tc.tile_pool(name="gath", bufs=2))
            psum = ctx.enter_context(tc.tile_pool(name="psum", bufs=6, space="PSUM"))
            psum64 = ctx.enter_context(tc.tile_pool(name="psum64", bufs=2, space="PSUM"))

            # ---- constants / weights (loaded once) ----
            def load(name, shape, dtype, src):
                t = consts.tile(shape, dtype, name=name)
                nc.sync.dma_start(out=t[:], in_=src[:])
                return t

            wg1 = load("wg1s", [128, 2, 256], F16, wg1_d)
            wp = load("wps", [128, 2, 512], F16, wp_d)
            wq = load("wqs", [128, 2, 512], F16, wq_d)
            wg2 = load("wg2s", [128, 4, 256], F16, wg2_d)
            wf1 = load("wf1s", [128, 2, 1024], F16, wf1_d)
            wf2 = load("wf2s", [128, 8, 256], F16, wf2_d)
            wb1 = load("wb1s", [128, 2, 64], F16, wb1_d)
            wb2 = load("wb2s", [64, 9, 64], F16, wb2_d)
            wb3 = load("wb3s", [64, 256], F16, wb3_d)
            bt1 = load("bt1s", [128, 2], F32, bt1_d)
            bt2 = load("bt2s", [128, 2], F32, bt2_d)
            bbp = load("bbps", [128, 4], F32, bbp_d)
            bbf1 = load("bbf1s", [128, 8], F32, bbf1_d)
            bbf2 = load("bbf2s", [128, 2], F32, bbf2_d)
            btb1 = load("btb1s", [64, 1], F32, btb1_d)
            btb2 = load("btb2s", [64, 1], F32, btb2_d)
            bsf = load("bsfs", [128, 2], F32, bsf_d)
            btf2 = load("btf2s", [128, 2], F32, btf2_d)

            ident = consts.tile([128, 128], F16, name="ident")
            make_identity(nc, ident[:])
            negid = consts.tile([128, 128], F16, name="negid")
            nc.scalar.activation(out=negid[:], in_=ident[:], func=AF.Copy,
                                 scale=NEG_BIG)
            ones = consts.tile([128, 128], F16, name="ones")
            nc.gpsimd.memset(ones[:], 1.0)
            # idbig[k, f] = 1 iff f == k + 384 (shifted identity for diag-kill)
            idbig = consts.tile([128, 1024], F16, name="idbig")
            nc.gpsimd.memset(idbig[:], 0.0)
            nc.gpsimd.affine_select(
                out=idbig[:], in_=idbig[:],
                compare_op=mybir.AluOpType.not_equal, fill=1.0,
                base=384, pattern=[[-1, 1024]], channel_multiplier=1)
            epsb = consts.tile([128, 1], F32, name="epsb")
            nc.gpsimd.memset(epsb[:], 1e-12)

            for img in range(IMGS_PER_CORE):
                P = pools[img]

                def T(tag, shape, dtype):
                    return P.tile(shape, dtype, name=f"{tag}{img}", tag=tag)

                # ==== load x (f16, prepacked on host) ====
                xh = T("xh", [128, 2, N], F16)
                for t in range(2):
                    nc.sync.dma_start(out=xh[:, t, :],
                                      in_=xh_d[img, t * 128:(t + 1) * 128, :])

                # ==== g1: featT [128, 2, N] f16 ====
                featT = T("feat", [128, 2, N], F16)
                for to in range(2):
                    pss = [psum.tile([128, 512], F32, name=f"ps_g1_{img}_{to}_{nb}", tag="ps")
                           for nb in range(2)]
                    for kt in range(2):
                        for nb in range(2):
                            nc.tensor.matmul(
                                pss[nb][:], lhsT=wg1[:, kt, to * 128:(to + 1) * 128],
                                rhs=xh[:, kt, nb * 512:(nb + 1) * 512],
                                start=(kt == 0), stop=(kt == 1))
                    for nb in range(2):
                        nc.scalar.activation(
                            out=featT[:, to, nb * 512:(nb + 1) * 512], in_=pss[nb][:],
                            func=AF.Identity, bias=bt1[:, to:to + 1])

                # ==== norms: invnb [128, N] f16 = rsqrt(n2) broadcast ====
                fsq = T("fx1", [128, 2, N], F16)
                for t in range(2):
                    nc.vector.tensor_mul(fsq[:, t, :], featT[:, t, :], featT[:, t, :])
                n2b = T("n2b", [128, N], F32)
                for nb in range(2):
                    psn = psum.tile([128, 512], F32, name=f"ps_n2_{img}_{nb}", tag="ps")
                    for kt in range(2):
                        nc.tensor.matmul(
                            psn[:], lhsT=ones[:],
                            rhs=fsq[:, kt, nb * 512:(nb + 1) * 512],
                            start=(kt == 0), stop=(kt == 1))
                    # n2 + eps, broadcast on all partitions
                    nc.scalar.activation(out=n2b[:, nb * 512:(nb + 1) * 512],
                                         in_=psn[:], func=AF.Identity,
                                         bias=epsb[:, 0:1])
                rn2 = T("fx1", [128, N], F32)  # reuses fsq slot (fsq dead)
                nc.vector.reciprocal_approx_fast(out=rn2[:], in_=n2b[:])
                invnb = T("invnb", [128, N], F16)
                nc.scalar.activation(out=invnb[:], in_=rn2[:], func=AF.Sqrt)
                xnT = T("n2b", [128, 2, N], F16)  # reuses n2b slot (n2b dead)
                for t in range(2):
                    nc.vector.tensor_mul(xnT[:, t, :], featT[:, t, :], invnb[:])

                # ==== sim scores per I-block + top-8 selection ====
                # ixbuf layout: [128, s(4), k(8), i(2)] so the wrap-DMA
                # source flattens to one contiguous run per partition.
                ixbuf = T("ix", [128, 4, 8, 2], dt.uint16)
                for I in range(8):
                    simblk = P.tile([128, N], F16, name=f"sim{img}_{I}",
                                    tag="sim", bufs=2)
                    pss = [psum.tile([128, 512], F32, name=f"ps_sim_{img}_{I}_{cb}", tag="ps")
                           for cb in range(2)]
                    for kt in range(2):
                        for cb in range(2):
                            has_diag = (cb == I // 4)
                            nc.tensor.matmul(
                                pss[cb][:], lhsT=featT[:, kt, I * 128:(I + 1) * 128],
                                rhs=xnT[:, kt, cb * 512:(cb + 1) * 512],
                                start=(kt == 0),
                                stop=(kt == 1 and not has_diag))
                    for cb in range(2):
                        if cb == I // 4:
                            off = I * 128 - cb * 512
                            nc.tensor.matmul(pss[cb][:], lhsT=negid[:],
                                             rhs=idbig[:, 384 - off:896 - off],
                                             start=False, stop=True)
                        nc.scalar.activation(
                            out=simblk[:, cb * 512:(cb + 1) * 512], in_=pss[cb][:],
                            func=AF.Copy)
                    mx = P.tile([128, 8], F16, name=f"mx{img}_{I}", tag="mx", bufs=2)
                    nc.vector.max(out=mx[:], in_=simblk[:])
                    nc.vector.max_index(out=ixbuf[:, I // 2, :, I % 2],
                                        in_max=mx[:], in_values=simblk[:])

                # ==== q [128 n-part, 8, 512] f16 (gather source) ====
                q_sb = T("q", [128, 8, 512], F16)
                q_dram = q_drams[img]
                for nt in range(8):
                    ps = psum.tile([128, 512], F32, name=f"ps_q_{img}_{nt}", tag="ps")
                    for kt in range(2):
                        nc.tensor.matmul(
                            ps[:], lhsT=featT[:, kt, nt * 128:(nt + 1) * 128],
                            rhs=wq[:, kt, :], start=(kt == 0), stop=(kt == 1))
                    nc.scalar.activation(out=q_sb[:, nt, :], in_=ps[:], func=AF.Copy)
                    if not sbuf_gather:
                        nc.sync.dma_start(out=q_dram[nt * 128:(nt + 1) * 128, :],
                                          in_=q_sb[:, nt, :])

                # ==== qT [128, 4, N] f16 (ch-major q, for the self term) ====
                qT = T("qT", [128, 4, N], F16)
                for a in range(4):
                    pss = [psum.tile([128, 512], F32, name=f"ps_qT_{img}_{a}_{nb}", tag="ps")
                           for nb in range(2)]
                    for kt in range(2):
                        for nb in range(2):
                            nc.tensor.matmul(
                                pss[nb][:], lhsT=wq[:, kt, a * 128:(a + 1) * 128],
                                rhs=featT[:, kt, nb * 512:(nb + 1) * 512],
                                start=(kt == 0), stop=(kt == 1))
                    for nb in range(2):
                        nc.scalar.activation(
                            out=qT[:, a, nb * 512:(nb + 1) * 512], in_=pss[nb][:],
                            func=AF.Copy)

                # ==== p^T [128, 4, N] f16 (ch-part, bias folded) ====
                pT = T("pT", [128, 4, N], F16)
                for a in range(4):
                    pss = [psum.tile([128, 512], F32, name=f"ps_p_{img}_{a}_{nb}", tag="ps")
                           for nb in range(2)]
                    for kt in range(2):
                        for nb in range(2):
                            nc.tensor.matmul(
                                pss[nb][:], lhsT=wp[:, kt, a * 128:(a + 1) * 128],
                                rhs=featT[:, kt, nb * 512:(nb + 1) * 512],
                                start=(kt == 0), stop=(kt == 1))
                    for nb in range(2):
                        nc.scalar.activation(
                            out=pT[:, a, nb * 512:(nb + 1) * 512], in_=pss[nb][:],
                            func=AF.Identity, bias=bbp[:, a:a + 1])

                # ==== wrapped idx buffer [128, 512] i16 ====
                # col = 128*s + 16*k + 8*i + g ; idx number within s-block
                # = 256*k + 128*i + 16*g + p16  -> node 256*s + 128*i + 16*g + p16
                wrapped = T("wrap", [128, 512], dt.int16)
                wview = wrapped[0:16, :].rearrange(
                    "p (s k i g) -> p s k i g", s=4, k=8, i=2, g=8)
                ixi = ixbuf[:].bitcast(dt.int16)
                for g in range(8):
                    nc.sync.dma_start(out=wview[:, :, :, :, g],
                                      in_=ixi[16 * g:16 * (g + 1), :, :, :])
                nc.sync.dma_start(out=wrapped[16:32, :], in_=wrapped[0:16, :])
                nc.sync.dma_start(out=wrapped[32:64, :], in_=wrapped[0:32, :])
                nc.sync.dma_start(out=wrapped[64:128, :], in_=wrapped[0:64, :])

                # ==== gather + 8-way max fold + self -> maxqT [128, 4, N] ====
                maxqT = T("maxq", [128, 4, N], F16)
                qflat = q_sb[:].rearrange("p a b -> p (a b)")
                for s in range(4):
                    go = pool_gath.tile([128, 4, 2048], F16, name=f"go{img}_{s}", tag="go")
                    if sbuf_gather:
                        nc.gpsimd.dma_gather(
                            out_ap=go[:], in_ap=qflat,
                            idxs_ap=wrapped[:, 128 * s:128 * (s + 1)],
                            num_idxs=2048, num_idxs_reg=2048, elem_size=512,
                            transpose=True, sbuf_tokens_per_rank=128,
                            sbuf_free_dim_per_rank=1024,
                            single_packet=False)
                    else:
                        nc.gpsimd.dma_gather(
                            out_ap=go[:], in_ap=q_dram[:],
                            idxs_ap=wrapped[:, 128 * s:128 * (s + 1)],
                            num_idxs=2048, num_idxs_reg=2048, elem_size=512,
                            transpose=True, single_packet=False)
                    gv = go[:].rearrange("p a (k n) -> p a k n", k=8)
                    sl = slice(256 * s, 256 * (s + 1))
                    nc.vector.tensor_max(gv[:, :, 4:8, :], gv[:, :, 0:4, :],
                                         gv[:, :, 4:8, :])
                    nc.vector.tensor_max(gv[:, :, 6:8, :], gv[:, :, 4:6, :],
                                         gv[:, :, 6:8, :])
                    nc.vector.tensor_max(gv[:, :, 7, :], gv[:, :, 6, :],
                                         gv[:, :, 7, :])
                    nc.vector.tensor_max(maxqT[:, :, sl], gv[:, :, 7, :],
                                         qT[:, :, sl])

                # ==== e = relu(p + maxq) [128, 4, N] f16 ====
                eT = T("q", [128, 4, N], F16)  # reuses q slot (q dead)
                nc.vector.tensor_add(eT[:], pT[:], maxqT[:])
                nc.vector.tensor_scalar_max(eT[:], eT[:], 0.0)

                # ==== g2 + residual (ident@xh in PSUM) -> hc f16 ====
                hc = T("hc", [128, 2, N], F16)
                for to in range(2):
                    pss = [psum.tile([128, 512], F32, name=f"ps_g2_{img}_{to}_{nb}", tag="ps")
                           for nb in range(2)]
                    for kt in range(4):
                        for nb in range(2):
                            nc.tensor.matmul(
                                pss[nb][:], lhsT=wg2[:, kt, to * 128:(to + 1) * 128],
                                rhs=eT[:, kt, nb * 512:(nb + 1) * 512],
                                start=(kt == 0), stop=False)
                    for nb in range(2):
                        nc.tensor.matmul(
                            pss[nb][:], lhsT=ident[:],
                            rhs=xh[:, to, nb * 512:(nb + 1) * 512],
                            start=False, stop=True)
                        nc.scalar.activation(
                            out=hc[:, to, nb * 512:(nb + 1) * 512], in_=pss[nb][:],
                            func=AF.Identity, bias=bt2[:, to:to + 1])

                # ==== FFN ====
                f1o = T("qT", [128, 8, N], F16)  # reuses qT slot (qT dead)
                for to in range(8):
                    pss = [psum.tile([128, 512], F32, name=f"ps_f1_{img}_{to}_{nb}", tag="ps")
                           for nb in range(2)]
                    for kt in range(2):
                        for nb in range(2):
                            nc.tensor.matmul(
                                pss[nb][:], lhsT=wf1[:, kt, to * 128:(to + 1) * 128],
                                rhs=hc[:, kt, nb * 512:(nb + 1) * 512],
                                start=(kt == 0), stop=(kt == 1))
                    for nb in range(2):
                        nc.scalar.activation(
                            out=f1o[:, to, nb * 512:(nb + 1) * 512], in_=pss[nb][:],
                            func=AF.Relu, bias=bbf1[:, to:to + 1])
                h2c = T("pT", [128, 2, N], F16)  # reuses pT slot (pT dead)
                for to in range(2):
                    pss = [psum.tile([128, 512], F32, name=f"ps_f2_{img}_{to}_{nb}", tag="ps")
                           for nb in range(2)]
                    for kt in range(8):
                        for nb in range(2):
                            nc.tensor.matmul(
                                pss[nb][:], lhsT=wf2[:, kt, to * 128:(to + 1) * 128],
                                rhs=f1o[:, kt, nb * 512:(nb + 1) * 512],
                                start=(kt == 0), stop=False)
                    for nb in range(2):
                        nc.tensor.matmul(
                            pss[nb][:], lhsT=ident[:],
                            rhs=hc[:, to, nb * 512:(nb + 1) * 512],
                            start=False, stop=True)
                        nc.scalar.activation(
                            out=h2c[:, to, nb * 512:(nb + 1) * 512], in_=pss[nb][:],
                            func=AF.Identity, bias=bbf2[:, to:to + 1])

                # ==== bottleneck ====
                pad = T("pad", [64, 34 * 34], F16)
                nc.vector.memset(pad[:], 0.0)
                pad3 = pad[:].rearrange("p (r c) -> p r c", r=34)
                for nb in range(2):
                    ps = psum64.tile([64, 512], F32, name=f"ps_b1_{img}_{nb}", tag="ps64")
                    for kt in range(2):
                        nc.tensor.matmul(
                            ps[:], lhsT=wb1[:, kt, :],
                            rhs=h2c[:, kt, nb * 512:(nb + 1) * 512],
                            start=(kt == 0), stop=(kt == 1))
                    # evacuate straight into the zero-padded conv input
                    nc.scalar.activation(
                        out=pad3[:, 1 + 16 * nb:17 + 16 * nb, 1:33],
                        in_=ps[:], func=AF.Relu, bias=btb1[:, 0:1])
                b2o = T("b2o", [64, N], F16)
                for nb in range(2):
                    ps = psum64.tile([64, 512], F32, name=f"ps_b2_{img}_{nb}", tag="ps64")
                    for tap in range(9):
                        dy, dx = tap // 3, tap % 3
                        rhs = pad3[:, 16 * nb + dy:16 * nb + dy + 16, dx:dx + 32]
                        nc.tensor.matmul(ps[:], lhsT=wb2[:, tap, :], rhs=rhs,
                                         start=(tap == 0), stop=(tap == 8))
                    nc.scalar.activation(out=b2o[:, nb * 512:(nb + 1) * 512],
                                         in_=ps[:], func=AF.Relu, bias=btb2[:, 0:1])

                # ==== b3 + h2 + x residuals in PSUM; final BN in evac ====
                out32 = T("maxq", [128, 2, N], F32)  # reuses maxq slot (dead)
                for to in range(2):
                    pss = [psum.tile([128, 512], F32, name=f"ps_b3_{img}_{to}_{nb}", tag="ps")
                           for nb in range(2)]
                    for nb in range(2):
                        sl = slice(nb * 512, (nb + 1) * 512)
                        nc.tensor.matmul(
                            pss[nb][:], lhsT=wb3[:, to * 128:(to + 1) * 128],
                            rhs=b2o[:, sl], start=True, stop=False)
                        nc.tensor.matmul(
                            pss[nb][:], lhsT=ident[:], rhs=h2c[:, to, sl],
                            start=False, stop=False)
                        nc.tensor.matmul(
                            pss[nb][:], lhsT=ident[:], rhs=xh[:, to, sl],
                            start=False, stop=True)
                        nc.scalar.activation(
                            out=out32[:, to, sl], in_=pss[nb][:],
                            func=AF.Identity, scale=bsf[:, to:to + 1],
                            bias=btf2[:, to:to + 1])
                    for nb in range(2):
                        nc.sync.dma_start(
                            out=out_d[img, to * 128:(to + 1) * 128,
                                      nb * 512:(nb + 1) * 512],
                            in_=out32[:, to, nb * 512:(nb + 1) * 512])

    nc.finalize()
    return nc


# --------------------------------------------------------------------------
# entry point
# --------------------------------------------------------------------------
def kernel(**inputs):
    inp = {k: np.asarray(v) for k, v in inputs.items()}
    w = _prep_weights(inp)

    if 'nc' not in _cache:
        _cache['nc'] = _build_bass()
    nc = _cache['nc']

    xh = inp['x'].astype(np.float16).reshape(B, C, N)
    in_maps = []
    for c in range(N_CORES):
        m = {'xh': np.ascontiguousarray(xh[c * 2:(c + 1) * 2])}
        m.update(w)
        in_maps.append(m)

    from concourse.bass_utils import run_bass_kernel_spmd
    trace = bool(os.environ.get("KBENCH_TRACE"))
    res = run_bass_kernel_spmd(nc, in_maps, core_ids=list(range(N_CORES)),
                               trace=trace)
    _cache['exec_time_ns'] = res.exec_time_ns
    _cache['results'] = res
    out = np.zeros((B, C, N), np.float32)
    for c in range(N_CORES):
        out[c * 2:(c + 1) * 2] = res.results[c]['out']
    return out.reshape(B, C, H, W)


# revision 8
# speedup vs baseline: 1.5959x; 1.5959x over previous
"""Trainium2 Bass kernel for nn_Block_17033840296551 (GNN message passing block).

Data-parallel over batch: 16 images -> 8 cores x 2 images. Each core runs the
full block (g1 conv -> kNN top-8+self -> EdgeConv max-agg -> g2 -> FFN ->
bottleneck -> final BN) on its 2 images with no cross-core communication.

v3 (gather + overlap rewrite):
  * NON-transposed dma_gather (n-major output): the transposed gather's rx
    side emits one 256B xbar-spray descriptor per 256 payload bytes (4x the
    descriptors), which made each 2k-idx gather ~20us of gpsimd descgen.
    n-major gathers are ~2x cheaper on the Q7s. The EdgeConv max-fold runs
    in n-major layout (where the self term is q_sb itself, no qT needed);
    e = relu(p+maxq) is then transposed to ch-major via 32 PE tile
    transposes for the g2 matmul.
  * Two-image software pipeline by construction: per-image tile pools AND
    per-image PSUM pools (a shared psum pool's slot rotation chained image
    1's first matmul behind image 0's last evacuation), with emission
    interleaved so image 1's compute phases sit inside image 0's gather
    window and vice versa.
  * All residual adds (g2, f2, b3, outer) ride the PSUM accumulation as
    identity matmuls; final BN fused into the last evacuation (scale/bias).
  * EdgeConv bias bp enters as a K=1 matmul (ones row x bp row) since it is
    per-channel and channels sit on the free axis in n-major layout.
  * Norms: n2 broadcast to 128 partitions via all-ones matmul, rsqrt via
    reciprocal_approx_fast + Sqrt activation.
"""

import os
import numpy as np

# problem constants (hardcoded per harness contract)
B, C, H, W = 16, 256, 32, 32
N = H * W           # 1024 pixels per image
K = 9
EPS = 1e-5
IMGS_PER_CORE = 2
N_CORES = 8
NEG_BIG = -30000.0

_cache = {}


# --------------------------------------------------------------------------
# host-side preprocessing
# --------------------------------------------------------------------------
def _bn_fold(p):
    g, b, m, v = np.asarray(p, np.float32)
    s = g / np.sqrt(v + EPS)
    t = b - m * s
    return s, t


def _pack_kxm(w_t, part=128):
    """[K, M] -> [part, K//part, M] (partition-major K tiling)."""
    Kd, M = w_t.shape
    kt = Kd // part
    return np.ascontiguousarray(w_t.reshape(kt, part, M).transpose(1, 0, 2))


def _pack_bias(b, part=128):
    n = b.shape[0]
    t = n // part
    return np.ascontiguousarray(b.reshape(t, part).T)  # [part, t]


def _prep_weights(inp):
    f16 = np.float16
    s1, t1 = _bn_fold(inp['g1_bn'])
    Wg1 = s1[:, None] * inp['g1_w']
    s2, t2 = _bn_fold(inp['g2_bn'])
    Wg2 = s2[:, None] * inp['g2_w']
    sf1, tf1 = _bn_fold(inp['f1_bn'])
    Wf1 = sf1[:, None] * inp['f1_w']
    bf1 = sf1 * inp['f1_b'] + tf1
    sf2, tf2 = _bn_fold(inp['f2_bn'])
    Wf2 = sf2[:, None] * inp['f2_w']
    bf2 = sf2 * inp['f2_b'] + tf2
    sb1, tb1 = _bn_fold(inp['b1_bn'])
    Wb1 = sb1[:, None] * inp['b1_w']
    sb2, tb2 = _bn_fold(inp['b2_bn'])
    Wb2 = sb2[:, None, None, None] * inp['b2_w']
    sb3, tb3 = _bn_fold(inp['b3_bn'])
    Wb3 = sb3[:, None] * inp['b3_w']
    sf, tf = _bn_fold(inp['bnf'])
    # final BN absorbs b3's bias: out = sf*(P + tb3) + tf, P = b3conv+h2+x
    btf2 = sf * tb3 + tf

    A = inp['edge_w'][:, :C]
    Bm = inp['edge_w'][:, C:]
    Wp = A - Bm
    Wq = Bm
    bp = inp['edge_b']

    wb2_t = np.zeros((64, 9, 64), f16)
    for dy in range(3):
        for dx in range(3):
            wb2_t[:, dy * 3 + dx, :] = Wb2[:, :, dy, dx].T.astype(f16)

    return {
        'wg1': _pack_kxm(Wg1.T.astype(f16)),           # [128,2,256]
        'wp': _pack_kxm(Wp.T.astype(f16)),             # [128,2,512]
        'wq': _pack_kxm(Wq.T.astype(f16)),             # [128,2,512]
        'wg2': _pack_kxm(Wg2.T.astype(f16)),           # [128,4,256]
        'wf1': _pack_kxm(Wf1.T.astype(f16)),           # [128,2,1024]
        'wf2': _pack_kxm(Wf2.T.astype(f16)),           # [128,8,256]
        'wb1': _pack_kxm(Wb1.T.astype(f16)),           # [128,2,64]
        'wb2': wb2_t,                                   # [64,9,64]
        'wb3': Wb3.T.astype(f16),                       # [64,256]
        'bt1': _pack_bias(t1),                          # [128,2] f32
        'bt2': _pack_bias(t2),
        'bpv': np.ascontiguousarray(bp.astype(f16).reshape(1, 512)),
        'bbf1': _pack_bias(bf1),                        # [128,8]
        'bbf2': _pack_bias(bf2),
        'btb1': np.ascontiguousarray(tb1[:, None].astype(np.float32)),  # [64,1]
        'btb2': np.ascontiguousarray(tb2[:, None].astype(np.float32)),
        'bsf': _pack_bias(sf),
        'btf2': _pack_bias(btf2),
    }


# --------------------------------------------------------------------------
# device kernel builder
# --------------------------------------------------------------------------
def _build_bass():
    import concourse.bass as bass
    import concourse.mybir as mybir
    from concourse import bacc
    from concourse.tile import TileContext
    from concourse.masks import make_identity

    dt = mybir.dt
    F16 = dt.float16
    F32 = dt.float32
    AF = mybir.ActivationFunctionType

    nc = bacc.Bacc()

    # ---- DRAM parameters ----
    xh_d = nc.declare_dram_parameter("xh", [IMGS_PER_CORE, C, N], F16, isOutput=False)
    wg1_d = nc.declare_dram_parameter("wg1", [128, 2, 256], F16, isOutput=False)
    wp_d = nc.declare_dram_parameter("wp", [128, 2, 512], F16, isOutput=False)
    wq_d = nc.declare_dram_parameter("wq", [128, 2, 512], F16, isOutput=False)
    wg2_d = nc.declare_dram_parameter("wg2", [128, 4, 256], F16, isOutput=False)
    wf1_d = nc.declare_dram_parameter("wf1", [128, 2, 1024], F16, isOutput=False)
    wf2_d = nc.declare_dram_parameter("wf2", [128, 8, 256], F16, isOutput=False)
    wb1_d = nc.declare_dram_parameter("wb1", [128, 2, 64], F16, isOutput=False)
    wb2_d = nc.declare_dram_parameter("wb2", [64, 9, 64], F16, isOutput=False)
    wb3_d = nc.declare_dram_parameter("wb3", [64, 256], F16, isOutput=False)
    bt1_d = nc.declare_dram_parameter("bt1", [128, 2], F32, isOutput=False)
    bt2_d = nc.declare_dram_parameter("bt2", [128, 2], F32, isOutput=False)
    bpv_d = nc.declare_dram_parameter("bpv", [1, 512], F16, isOutput=False)
    bbf1_d = nc.declare_dram_parameter("bbf1", [128, 8], F32, isOutput=False)
    bbf2_d = nc.declare_dram_parameter("bbf2", [128, 2], F32, isOutput=False)
    btb1_d = nc.declare_dram_parameter("btb1", [64, 1], F32, isOutput=False)
    btb2_d = nc.declare_dram_parameter("btb2", [64, 1], F32, isOutput=False)
    bsf_d = nc.declare_dram_parameter("bsf", [128, 2], F32, isOutput=False)
    btf2_d = nc.declare_dram_parameter("btf2", [128, 2], F32, isOutput=False)
    q_drams = [nc.dram_tensor(f"q_dram{i}", [N, 512], F16)
               for i in range(IMGS_PER_CORE)]
    out_d = nc.declare_dram_parameter("out", [IMGS_PER_CORE, C, N], F32, isOutput=True)

    with TileContext(nc) as tc:
        import contextlib
        ctx = contextlib.ExitStack()
        with ctx:
            consts = ctx.enter_context(tc.tile_pool(name="consts", bufs=1))
            # per-image pools: every tag is a dedicated buffer; PSUM pools are
            # per-image so slot rotation never chains one image behind the other
            pools = [ctx.enter_context(tc.tile_pool(name=f"im{i}", bufs=1))
                     for i in range(IMGS_PER_CORE)]
            psums = [ctx.enter_context(
                tc.tile_pool(name=f"psum{i}", bufs=3, space="PSUM"))
                for i in range(IMGS_PER_CORE)]
            psum64s = [ctx.enter_context(
                tc.tile_pool(name=f"psum64_{i}", bufs=1, space="PSUM"))
                for i in range(IMGS_PER_CORE)]
            pool_gath = ctx.enter_context(tc.tile_pool(name="gath", bufs=2))

            # ---- constants / weights (loaded once) ----
            def load(name, shape, dtype, src):
                t = consts.tile(shape, dtype, name=name)
                nc.sync.dma_start(out=t[:], in_=src[:])
                return t

            wg1 = load("wg1s", [128, 2, 256], F16, wg1_d)
            wp = load("wps", [128, 2, 512], F16, wp_d)
            wq = load("wqs", [128, 2, 512], F16, wq_d)
            wg2 = load("wg2s", [128, 4, 256], F16, wg2_d)
            wf1 = load("wf1s", [128, 2, 1024], F16, wf1_d)
            wf2 = load("wf2s", [128, 8, 256], F16, wf2_d)
            wb1 = load("wb1s", [128, 2, 64], F16, wb1_d)
            wb2 = load("wb2s", [64, 9, 64], F16, wb2_d)
            wb3 = load("wb3s", [64, 256], F16, wb3_d)
            bt1 = load("bt1s", [128, 2], F32, bt1_d)
            bt2 = load("bt2s", [128, 2], F32, bt2_d)
            bpv = load("bpvs", [1, 512], F16, bpv_d)
            bbf1 = load("bbf1s", [128, 8], F32, bbf1_d)
            bbf2 = load("bbf2s", [128, 2], F32, bbf2_d)
            btb1 = load("btb1s", [64, 1], F32, btb1_d)
            btb2 = load("btb2s", [64, 1], F32, btb2_d)
            bsf = load("bsfs", [128, 2], F32, bsf_d)
            btf2 = load("btf2s", [128, 2], F32, btf2_d)

            ident = consts.tile([128, 128], F16, name="ident")
            make_identity(nc, ident[:])
            negid = consts.tile([128, 128], F16, name="negid")
            nc.scalar.activation(out=negid[:], in_=ident[:], func=AF.Copy,
                                 scale=NEG_BIG)
            ones = consts.tile([128, 128], F16, name="ones")
            nc.gpsimd.memset(ones[:], 1.0)
            # idbig[k, f] = 1 iff f == k + 384 (shifted identity for diag-kill)
            idbig = consts.tile([128, 1024], F16, name="idbig")
            nc.gpsimd.memset(idbig[:], 0.0)
            nc.gpsimd.affine_select(
                out=idbig[:], in_=idbig[:],
                compare_op=mybir.AluOpType.not_equal, fill=1.0,
                base=384, pattern=[[-1, 1024]], channel_multiplier=1)
            epsb = consts.tile([128, 1], F32, name="epsb")
            nc.gpsimd.memset(epsb[:], 1e-12)

            # per-image tile state
            st = [{} for _ in range(IMGS_PER_CORE)]

            def T(img, tag, shape, dtype):
                t = pools[img].tile(shape, dtype, name=f"{tag}_{img}", tag=tag)
                st[img][tag] = t
                return t

            def phase_pre(img):
                """load x, g1 conv, feature norms."""
                xh = T(img, "xh", [128, 2, N], F16)
                for t in range(2):
                    nc.sync.dma_start(out=xh[:, t, :],
                                      in_=xh_d[img, t * 128:(t + 1) * 128, :])
                featT = T(img, "feat", [128, 2, N], F16)
                for to in range(2):
                    pss = [psums[img].tile([128, 512], F32,
                                           name=f"ps_g1_{img}_{to}_{nb}", tag="ps")
                           for nb in range(2)]
                    for kt in range(2):
                        for nb in range(2):
                            nc.tensor.matmul(
                                pss[nb][:], lhsT=wg1[:, kt, to * 128:(to + 1) * 128],
                                rhs=xh[:, kt, nb * 512:(nb + 1) * 512],
                                start=(kt == 0), stop=(kt == 1))
                    for nb in range(2):
                        nc.scalar.activation(
                            out=featT[:, to, nb * 512:(nb + 1) * 512], in_=pss[nb][:],
                            func=AF.Identity, bias=bt1[:, to:to + 1])
                fsq = T(img, "fx1", [128, 2, N], F16)
                for t in range(2):
                    nc.vector.tensor_mul(fsq[:, t, :], featT[:, t, :], featT[:, t, :])
                n2b = T(img, "n2b", [128, N], F32)
                for nb in range(2):
                    psn = psums[img].tile([128, 512], F32,
                                          name=f"ps_n2_{img}_{nb}", tag="ps")
                    for kt in range(2):
                        nc.tensor.matmul(
                            psn[:], lhsT=ones[:],
                            rhs=fsq[:, kt, nb * 512:(nb + 1) * 512],
                            start=(kt == 0), stop=(kt == 1))
                    nc.scalar.activation(out=n2b[:, nb * 512:(nb + 1) * 512],
                                         in_=psn[:], func=AF.Identity,
                                         bias=epsb[:, 0:1])
                rn2 = T(img, "fx1", [128, N], F32)  # reuses fsq slot (fsq dead)
                nc.vector.reciprocal_approx_fast(out=rn2[:], in_=n2b[:])
                invnb = T(img, "invnb", [128, N], F16)
                nc.scalar.activation(out=invnb[:], in_=rn2[:], func=AF.Sqrt)
                xnT = T(img, "n2b", [128, 2, N], F16)  # reuses n2b slot
                for t in range(2):
                    nc.vector.tensor_mul(xnT[:, t, :], featT[:, t, :], invnb[:])

            def phase_sel(img, Is):
                """sim I-blocks + top-8 selection for I in Is."""
                featT = st[img]["feat"]
                xnT = st[img]["n2b"]
                if 0 in Is:
                    st[img]["ixt"] = T(img, "ix", [128, 4, 8, 2], dt.uint16)
                ixbuf = st[img]["ixt"]
                for I in Is:
                    simblk = pools[img].tile([128, N], F16, name=f"sim{img}_{I}",
                                             tag="sim", bufs=2)
                    pss = [psums[img].tile([128, 512], F32,
                                           name=f"ps_sim_{img}_{I}_{cb}", tag="ps")
                           for cb in range(2)]
                    for kt in range(2):
                        for cb in range(2):
                            has_diag = (cb == I // 4)
                            nc.tensor.matmul(
                                pss[cb][:], lhsT=featT[:, kt, I * 128:(I + 1) * 128],
                                rhs=xnT[:, kt, cb * 512:(cb + 1) * 512],
                                start=(kt == 0),
                                stop=(kt == 1 and not has_diag))
                    for cb in range(2):
                        if cb == I // 4:
                            off = I * 128 - cb * 512
                            nc.tensor.matmul(pss[cb][:], lhsT=negid[:],
                                             rhs=idbig[:, 384 - off:896 - off],
                                             start=False, stop=True)
                        nc.scalar.activation(
                            out=simblk[:, cb * 512:(cb + 1) * 512], in_=pss[cb][:],
                            func=AF.Copy)
                    mx = pools[img].tile([128, 8], F16, name=f"mx{img}_{I}",
                                         tag="mx", bufs=2)
                    nc.vector.max(out=mx[:], in_=simblk[:])
                    nc.vector.max_index(out=ixbuf[:, I // 2, :, I % 2],
                                        in_max=mx[:], in_values=simblk[:])

            def phase_wrap(img):
                """wrapped idx buffer [128, 512] i16.
                col = 128*s + 16*k + 8*i + g ; idx number within s-block
                = 256*k + 128*i + 16*g + p16 -> node 256*s + 128*i + 16*g + p16
                """
                wrapped = T(img, "wrap", [128, 512], dt.int16)
                wview = wrapped[0:16, :].rearrange(
                    "p (s k i g) -> p s k i g", s=4, k=8, i=2, g=8)
                ixi = st[img]["ix"][:].bitcast(dt.int16)
                for g in range(8):
                    nc.sync.dma_start(out=wview[:, :, :, :, g],
                                      in_=ixi[16 * g:16 * (g + 1), :, :, :])
                nc.sync.dma_start(out=wrapped[16:32, :], in_=wrapped[0:16, :])
                nc.sync.dma_start(out=wrapped[32:64, :], in_=wrapped[0:32, :])
                nc.sync.dma_start(out=wrapped[64:128, :], in_=wrapped[0:64, :])

            def phase_qp(img):
                """q (n-major, to DRAM for gather) and p (n-major, +bp bias)."""
                featT = st[img]["feat"]
                q_sb = T(img, "q", [128, 8, 512], F16)
                q_dram = q_drams[img]
                for nt in range(8):
                    ps = psums[img].tile([128, 512], F32,
                                         name=f"ps_q_{img}_{nt}", tag="ps")
                    for kt in range(2):
                        nc.tensor.matmul(
                            ps[:], lhsT=featT[:, kt, nt * 128:(nt + 1) * 128],
                            rhs=wq[:, kt, :], start=(kt == 0), stop=(kt == 1))
                    nc.scalar.activation(out=q_sb[:, nt, :], in_=ps[:], func=AF.Copy)
                    nc.sync.dma_start(out=q_dram[nt * 128:(nt + 1) * 128, :],
                                      in_=q_sb[:, nt, :])
                p_nm = T(img, "pT", [128, 8, 512], F16)
                for nt in range(8):
                    ps = psums[img].tile([128, 512], F32,
                                         name=f"ps_p_{img}_{nt}", tag="ps")
                    for kt in range(2):
                        nc.tensor.matmul(
                            ps[:], lhsT=featT[:, kt, nt * 128:(nt + 1) * 128],
                            rhs=wp[:, kt, :], start=(kt == 0), stop=False)
                    # per-channel EdgeConv bias via K=1 broadcast matmul
                    nc.tensor.matmul(ps[:], lhsT=ones[0:1, :], rhs=bpv[0:1, :],
                                     start=False, stop=True)
                    nc.scalar.activation(out=p_nm[:, nt, :], in_=ps[:], func=AF.Copy)

            def phase_gather(img, s):
                """gather neighbors for s-block (n-major) + 8-way fold + self."""
                if s == 0:
                    st[img]["maxqt"] = T(img, "maxq", [128, 8, 512], F16)
                maxq = st[img]["maxqt"]
                q_sb = st[img]["q"]
                wrapped = st[img]["wrap"]
                go = pool_gath.tile([128, 16, 512], F16, name=f"go{img}_{s}",
                                    tag="go")
                nc.gpsimd.dma_gather(
                    out_ap=go[:], in_ap=q_drams[img][:],
                    idxs_ap=wrapped[:, 128 * s:128 * (s + 1)],
                    num_idxs=2048, num_idxs_reg=2048, elem_size=512,
                    transpose=False, single_packet=False)
                gv = go[:].rearrange("p (k h) c -> p k h c", k=8)
                nc.vector.tensor_max(gv[:, 4:8, :, :], gv[:, 0:4, :, :],
                                     gv[:, 4:8, :, :])
                nc.vector.tensor_max(gv[:, 6:8, :, :], gv[:, 4:6, :, :],
                                     gv[:, 6:8, :, :])
                nc.vector.tensor_max(gv[:, 7, :, :], gv[:, 6, :, :],
                                     gv[:, 7, :, :])
                nc.vector.tensor_max(maxq[:, 2 * s:2 * s + 2, :],
                                     gv[:, 7, :, :], q_sb[:, 2 * s:2 * s + 2, :])

            def phase_edge(img):
                """e = relu(p + maxq) in n-major, then PE-transpose to
                ch-major eT [128, 4, N]."""
                p_nm = st[img]["pT"]
                maxq = st[img]["maxqt"]
                e_nm = T(img, "n2b", [128, 8, 512], F16)  # reuses xnT slot
                nc.vector.tensor_add(e_nm[:], p_nm[:], maxq[:])
                nc.vector.tensor_scalar_max(e_nm[:], e_nm[:], 0.0)
                eT = T(img, "q", [128, 4, N], F16)  # reuses q slot (q dead)
                for a in range(4):
                    for nb in range(2):
                        ps = psums[img].tile([128, 512], F16,
                                             name=f"ps_tr_{img}_{a}_{nb}", tag="ps")
                        for j in range(4):
                            nc.tensor.transpose(
                                out=ps[:, j * 128:(j + 1) * 128],
                                in_=e_nm[:, 4 * nb + j, a * 128:(a + 1) * 128],
                                identity=ident[:])
                        nc.scalar.activation(
                            out=eT[:, a, nb * 512:(nb + 1) * 512], in_=ps[:],
                            func=AF.Copy)

            def phase_g2(img):
                """g2 conv + residual (ident@xh in PSUM) -> hc f16."""
                eT = st[img]["q"]
                xh = st[img]["xh"]
                hc = T(img, "hc", [128, 2, N], F16)
                for to in range(2):
                    pss = [psums[img].tile([128, 512], F32,
                                           name=f"ps_g2_{img}_{to}_{nb}", tag="ps")
                           for nb in range(2)]
                    for kt in range(4):
                        for nb in range(2):
                            nc.tensor.matmul(
                                pss[nb][:], lhsT=wg2[:, kt, to * 128:(to + 1) * 128],
                                rhs=eT[:, kt, nb * 512:(nb + 1) * 512],
                                start=(kt == 0), stop=False)
                    for nb in range(2):
                        nc.tensor.matmul(
                            pss[nb][:], lhsT=ident[:],
                            rhs=xh[:, to, nb * 512:(nb + 1) * 512],
                            start=False, stop=True)
                        nc.scalar.activation(
                            out=hc[:, to, nb * 512:(nb + 1) * 512], in_=pss[nb][:],
                            func=AF.Identity, bias=bt2[:, to:to + 1])

            def phase_f1(img):
                hc = st[img]["hc"]
                f1o = T(img, "f1o", [128, 8, N], F16)
                for to in range(8):
                    pss = [psums[img].tile([128, 512], F32,
                                           name=f"ps_f1_{img}_{to}_{nb}", tag="ps")
                           for nb in range(2)]
                    for kt in range(2):
                        for nb in range(2):
                            nc.tensor.matmul(
                                pss[nb][:], lhsT=wf1[:, kt, to * 128:(to + 1) * 128],
                                rhs=hc[:, kt, nb * 512:(nb + 1) * 512],
                                start=(kt == 0), stop=(kt == 1))
                    for nb in range(2):
                        nc.scalar.activation(
                            out=f1o[:, to, nb * 512:(nb + 1) * 512], in_=pss[nb][:],
                            func=AF.Relu, bias=bbf1[:, to:to + 1])

            def phase_f2(img):
                f1o = st[img]["f1o"]
                hc = st[img]["hc"]
                h2c = T(img, "pT", [128, 2, N], F16)  # reuses p slot (p dead)
                for to in range(2):
                    pss = [psums[img].tile([128, 512], F32,
                                           name=f"ps_f2_{img}_{to}_{nb}", tag="ps")
                           for nb in range(2)]
                    for kt in range(8):
                        for nb in range(2):
                            nc.tensor.matmul(
                                pss[nb][:], lhsT=wf2[:, kt, to * 128:(to + 1) * 128],
                                rhs=f1o[:, kt, nb * 512:(nb + 1) * 512],
                                start=(kt == 0), stop=False)
                    for nb in range(2):
                        nc.tensor.matmul(
                            pss[nb][:], lhsT=ident[:],
                            rhs=hc[:, to, nb * 512:(nb + 1) * 512],
                            start=False, stop=True)
                        nc.scalar.activation(
                            out=h2c[:, to, nb * 512:(nb + 1) * 512], in_=pss[nb][:],
                            func=AF.Identity, bias=bbf2[:, to:to + 1])

            def phase_bneck(img):
                h2c = st[img]["pT"]
                pad = T(img, "pad", [64, 34 * 34], F16)
                nc.vector.memset(pad[:], 0.0)
                pad3 = pad[:].rearrange("p (r c) -> p r c", r=34)
                for nb in range(2):
                    ps = psum64s[img].tile([64, 512], F32,
                                           name=f"ps_b1_{img}_{nb}", tag="ps64")
                    for kt in range(2):
                        nc.tensor.matmul(
                            ps[:], lhsT=wb1[:, kt, :],
                            rhs=h2c[:, kt, nb * 512:(nb + 1) * 512],
                            start=(kt == 0), stop=(kt == 1))
                    # evacuate straight into the zero-padded conv input
                    nc.scalar.activation(
                        out=pad3[:, 1 + 16 * nb:17 + 16 * nb, 1:33],
                        in_=ps[:].rearrange("p (r c) -> p r c", r=16),
                        func=AF.Relu, bias=btb1[:, 0:1])
                b2o = T(img, "b2o", [64, N], F16)
                for nb in range(2):
                    ps = psum64s[img].tile([64, 512], F32,
                                           name=f"ps_b2_{img}_{nb}", tag="ps64")
                    for tap in range(9):
                        dy, dx = tap // 3, tap % 3
                        rhs = pad3[:, 16 * nb + dy:16 * nb + dy + 16, dx:dx + 32]
                        nc.tensor.matmul(ps[:], lhsT=wb2[:, tap, :], rhs=rhs,
                                         start=(tap == 0), stop=(tap == 8))
                    nc.scalar.activation(out=b2o[:, nb * 512:(nb + 1) * 512],
                                         in_=ps[:], func=AF.Relu, bias=btb2[:, 0:1])

            def phase_out(img):
                """b3 + h2 + x residuals in PSUM; final BN in the evacuation."""
                h2c = st[img]["pT"]
                xh = st[img]["xh"]
                b2o = st[img]["b2o"]
                out32 = T(img, "maxq", [128, 2, N], F32)  # reuses maxq slot
                for to in range(2):
                    pss = [psums[img].tile([128, 512], F32,
                                           name=f"ps_b3_{img}_{to}_{nb}", tag="ps")
                           for nb in range(2)]
                    for nb in range(2):
                        sl = slice(nb * 512, (nb + 1) * 512)
                        nc.tensor.matmul(
                            pss[nb][:], lhsT=wb3[:, to * 128:(to + 1) * 128],
                            rhs=b2o[:, sl], start=True, stop=False)
                        nc.tensor.matmul(
                            pss[nb][:], lhsT=ident[:], rhs=h2c[:, to, sl],
                            start=False, stop=False)
                        nc.tensor.matmul(
                            pss[nb][:], lhsT=ident[:], rhs=xh[:, to, sl],
                            start=False, stop=True)
                        nc.scalar.activation(
                            out=out32[:, to, sl], in_=pss[nb][:],
                            func=AF.Identity, scale=bsf[:, to:to + 1],
                            bias=btf2[:, to:to + 1])
                    for nb in range(2):
                        nc.sync.dma_start(
                            out=out_d[img, to * 128:(to + 1) * 128,
                                      nb * 512:(nb + 1) * 512],
                            in_=out32[:, to, nb * 512:(nb + 1) * 512])

            # ---- two-image software pipeline (emission order = priority) ----
            phase_pre(0)
            phase_sel(0, range(8))
            phase_wrap(0)
            phase_qp(0)
            phase_gather(0, 0)
            phase_pre(1)
            phase_gather(0, 1)
            phase_sel(1, range(0, 4))
            phase_gather(0, 2)
            phase_sel(1, range(4, 8))
            phase_gather(0, 3)
            phase_wrap(1)
            phase_qp(1)
            phase_edge(0)
            phase_g2(0)
            phase_gather(1, 0)
            phase_f1(0)
            phase_gather(1, 1)
            phase_f2(0)
            phase_gather(1, 2)
            phase_bneck(0)
            phase_gather(1, 3)
            phase_out(0)
            phase_edge(1)
            phase_g2(1)
            phase_f1(1)
            phase_f2(1)
            phase_bneck(1)
            phase_out(1)

    nc.finalize()
    return nc


# --------------------------------------------------------------------------
# entry point
# --------------------------------------------------------------------------
def kernel(**inputs):
    inp = {k: np.asarray(v) for k, v in inputs.items()}
    w = _prep_weights(inp)

    if 'nc' not in _cache:
        _cache['nc'] = _build_bass()
    nc = _cache['nc']

    xh = inp['x'].astype(np.float16).reshape(B, C, N)
    in_maps = []
    for c in range(N_CORES):
        m = {'xh': np.ascontiguousarray(xh[c * 2:(c + 1) * 2])}
        m.update(w)
        in_maps.append(m)

    from concourse.bass_utils import run_bass_kernel_spmd
    trace = bool(os.environ.get("KBENCH_TRACE"))
    res = run_bass_kernel_spmd(nc, in_maps, core_ids=list(range(N_CORES)),
                               trace=trace)
    _cache['exec_time_ns'] = res.exec_time_ns
    _cache['results'] = res
    out = np.zeros((B, C, N), np.float32)
    for c in range(N_CORES):
        out[c * 2:(c + 1) * 2] = res.results[c]['out']
    return out.reshape(B, C, H, W)


# revision 11
# speedup vs baseline: 1.6017x; 1.0036x over previous
"""Trainium2 Bass kernel for nn_Block_17033840296551 (GNN message passing block).

Data-parallel over batch: 16 images -> 8 cores x 2 images. Each core runs the
full block (g1 conv -> kNN top-8+self -> EdgeConv max-agg -> g2 -> FFN ->
bottleneck -> final BN) on its 2 images with no cross-core communication.

v3 (gather + overlap rewrite):
  * NON-transposed dma_gather (n-major output): the transposed gather's rx
    side emits one 256B xbar-spray descriptor per 256 payload bytes (4x the
    descriptors), which made each 2k-idx gather ~20us of gpsimd descgen.
    n-major gathers are ~2x cheaper on the Q7s. The EdgeConv max-fold runs
    in n-major layout (where the self term is q_sb itself, no qT needed);
    e = relu(p+maxq) is then transposed to ch-major via 32 PE tile
    transposes for the g2 matmul.
  * Two-image software pipeline by construction: per-image tile pools AND
    per-image PSUM pools (a shared psum pool's slot rotation chained image
    1's first matmul behind image 0's last evacuation), with emission
    interleaved so image 1's compute phases sit inside image 0's gather
    window and vice versa.
  * All residual adds (g2, f2, b3, outer) ride the PSUM accumulation as
    identity matmuls; final BN fused into the last evacuation (scale/bias).
  * EdgeConv bias bp enters as a K=1 matmul (ones row x bp row) since it is
    per-channel and channels sit on the free axis in n-major layout.
  * Norms: n2 broadcast to 128 partitions via all-ones matmul, rsqrt via
    reciprocal_approx_fast + Sqrt activation.
"""

import os
import numpy as np

# problem constants (hardcoded per harness contract)
B, C, H, W = 16, 256, 32, 32
N = H * W           # 1024 pixels per image
K = 9
EPS = 1e-5
IMGS_PER_CORE = 2
N_CORES = 8
NEG_BIG = -30000.0

_cache = {}


# --------------------------------------------------------------------------
# host-side preprocessing
# --------------------------------------------------------------------------
def _bn_fold(p):
    g, b, m, v = np.asarray(p, np.float32)
    s = g / np.sqrt(v + EPS)
    t = b - m * s
    return s, t


def _pack_kxm(w_t, part=128):
    """[K, M] -> [part, K//part, M] (partition-major K tiling)."""
    Kd, M = w_t.shape
    kt = Kd // part
    return np.ascontiguousarray(w_t.reshape(kt, part, M).transpose(1, 0, 2))


def _pack_bias(b, part=128):
    n = b.shape[0]
    t = n // part
    return np.ascontiguousarray(b.reshape(t, part).T)  # [part, t]


def _prep_weights(inp):
    f16 = np.float16
    s1, t1 = _bn_fold(inp['g1_bn'])
    Wg1 = s1[:, None] * inp['g1_w']
    s2, t2 = _bn_fold(inp['g2_bn'])
    Wg2 = s2[:, None] * inp['g2_w']
    sf1, tf1 = _bn_fold(inp['f1_bn'])
    Wf1 = sf1[:, None] * inp['f1_w']
    bf1 = sf1 * inp['f1_b'] + tf1
    sf2, tf2 = _bn_fold(inp['f2_bn'])
    Wf2 = sf2[:, None] * inp['f2_w']
    bf2 = sf2 * inp['f2_b'] + tf2
    sb1, tb1 = _bn_fold(inp['b1_bn'])
    Wb1 = sb1[:, None] * inp['b1_w']
    sb2, tb2 = _bn_fold(inp['b2_bn'])
    Wb2 = sb2[:, None, None, None] * inp['b2_w']
    sb3, tb3 = _bn_fold(inp['b3_bn'])
    Wb3 = sb3[:, None] * inp['b3_w']
    sf, tf = _bn_fold(inp['bnf'])
    # final BN absorbs b3's bias: out = sf*(P + tb3) + tf, P = b3conv+h2+x
    btf2 = sf * tb3 + tf

    A = inp['edge_w'][:, :C]
    Bm = inp['edge_w'][:, C:]
    Wp = A - Bm
    Wq = Bm
    bp = inp['edge_b']

    wb2_t = np.zeros((64, 9, 64), f16)
    for dy in range(3):
        for dx in range(3):
            wb2_t[:, dy * 3 + dx, :] = Wb2[:, :, dy, dx].T.astype(f16)

    return {
        'wg1': _pack_kxm(Wg1.T.astype(f16)),           # [128,2,256]
        'wp': _pack_kxm(Wp.T.astype(f16)),             # [128,2,512]
        'wq': _pack_kxm(Wq.T.astype(f16)),             # [128,2,512]
        'wg2': _pack_kxm(Wg2.T.astype(f16)),           # [128,4,256]
        'wf1': _pack_kxm(Wf1.T.astype(f16)),           # [128,2,1024]
        'wf2': _pack_kxm(Wf2.T.astype(f16)),           # [128,8,256]
        'wb1': _pack_kxm(Wb1.T.astype(f16)),           # [128,2,64]
        'wb2': wb2_t,                                   # [64,9,64]
        'wb3': Wb3.T.astype(f16),                       # [64,256]
        'bt1': _pack_bias(t1),                          # [128,2] f32
        'bt2': _pack_bias(t2),
        'bpv': np.ascontiguousarray(bp.astype(f16).reshape(1, 512)),
        'bbf1': _pack_bias(bf1),                        # [128,8]
        'bbf2': _pack_bias(bf2),
        'btb1': np.ascontiguousarray(tb1[:, None].astype(np.float32)),  # [64,1]
        'btb2': np.ascontiguousarray(tb2[:, None].astype(np.float32)),
        'bsf': _pack_bias(sf),
        'btf2': _pack_bias(btf2),
    }


# --------------------------------------------------------------------------
# device kernel builder
# --------------------------------------------------------------------------
def _build_bass():
    import concourse.bass as bass
    import concourse.mybir as mybir
    from concourse import bacc
    from concourse.tile import TileContext
    from concourse.masks import make_identity

    dt = mybir.dt
    F16 = dt.float16
    F32 = dt.float32
    AF = mybir.ActivationFunctionType

    nc = bacc.Bacc()

    # ---- DRAM parameters ----
    xh_d = nc.declare_dram_parameter("xh", [IMGS_PER_CORE, C, N], F16, isOutput=False)
    wg1_d = nc.declare_dram_parameter("wg1", [128, 2, 256], F16, isOutput=False)
    wp_d = nc.declare_dram_parameter("wp", [128, 2, 512], F16, isOutput=False)
    wq_d = nc.declare_dram_parameter("wq", [128, 2, 512], F16, isOutput=False)
    wg2_d = nc.declare_dram_parameter("wg2", [128, 4, 256], F16, isOutput=False)
    wf1_d = nc.declare_dram_parameter("wf1", [128, 2, 1024], F16, isOutput=False)
    wf2_d = nc.declare_dram_parameter("wf2", [128, 8, 256], F16, isOutput=False)
    wb1_d = nc.declare_dram_parameter("wb1", [128, 2, 64], F16, isOutput=False)
    wb2_d = nc.declare_dram_parameter("wb2", [64, 9, 64], F16, isOutput=False)
    wb3_d = nc.declare_dram_parameter("wb3", [64, 256], F16, isOutput=False)
    bt1_d = nc.declare_dram_parameter("bt1", [128, 2], F32, isOutput=False)
    bt2_d = nc.declare_dram_parameter("bt2", [128, 2], F32, isOutput=False)
    bpv_d = nc.declare_dram_parameter("bpv", [1, 512], F16, isOutput=False)
    bbf1_d = nc.declare_dram_parameter("bbf1", [128, 8], F32, isOutput=False)
    bbf2_d = nc.declare_dram_parameter("bbf2", [128, 2], F32, isOutput=False)
    btb1_d = nc.declare_dram_parameter("btb1", [64, 1], F32, isOutput=False)
    btb2_d = nc.declare_dram_parameter("btb2", [64, 1], F32, isOutput=False)
    bsf_d = nc.declare_dram_parameter("bsf", [128, 2], F32, isOutput=False)
    btf2_d = nc.declare_dram_parameter("btf2", [128, 2], F32, isOutput=False)
    q_drams = [nc.dram_tensor(f"q_dram{i}", [N, 512], F16)
               for i in range(IMGS_PER_CORE)]
    out_d = nc.declare_dram_parameter("out", [IMGS_PER_CORE, C, N], F32, isOutput=True)

    with TileContext(nc) as tc:
        import contextlib
        ctx = contextlib.ExitStack()
        with ctx:
            consts = ctx.enter_context(tc.tile_pool(name="consts", bufs=1))
            # per-image pools: every tag is a dedicated buffer; PSUM pools are
            # per-image so slot rotation never chains one image behind the other
            pools = [ctx.enter_context(tc.tile_pool(name=f"im{i}", bufs=1))
                     for i in range(IMGS_PER_CORE)]
            psums = [ctx.enter_context(
                tc.tile_pool(name=f"psum{i}", bufs=3, space="PSUM"))
                for i in range(IMGS_PER_CORE)]
            psum64s = [ctx.enter_context(
                tc.tile_pool(name=f"psum64_{i}", bufs=1, space="PSUM"))
                for i in range(IMGS_PER_CORE)]
            pool_gath = ctx.enter_context(tc.tile_pool(name="gath", bufs=2))

            # ---- constants / weights (loaded once) ----
            def load(name, shape, dtype, src):
                t = consts.tile(shape, dtype, name=name)
                nc.sync.dma_start(out=t[:], in_=src[:])
                return t

            wg1 = load("wg1s", [128, 2, 256], F16, wg1_d)
            wp = load("wps", [128, 2, 512], F16, wp_d)
            wq = load("wqs", [128, 2, 512], F16, wq_d)
            wg2 = load("wg2s", [128, 4, 256], F16, wg2_d)
            wf1 = load("wf1s", [128, 2, 1024], F16, wf1_d)
            wf2 = load("wf2s", [128, 8, 256], F16, wf2_d)
            wb1 = load("wb1s", [128, 2, 64], F16, wb1_d)
            wb2 = load("wb2s", [64, 9, 64], F16, wb2_d)
            wb3 = load("wb3s", [64, 256], F16, wb3_d)
            bt1 = load("bt1s", [128, 2], F32, bt1_d)
            bt2 = load("bt2s", [128, 2], F32, bt2_d)
            bpv = load("bpvs", [1, 512], F16, bpv_d)
            bbf1 = load("bbf1s", [128, 8], F32, bbf1_d)
            bbf2 = load("bbf2s", [128, 2], F32, bbf2_d)
            btb1 = load("btb1s", [64, 1], F32, btb1_d)
            btb2 = load("btb2s", [64, 1], F32, btb2_d)
            bsf = load("bsfs", [128, 2], F32, bsf_d)
            btf2 = load("btf2s", [128, 2], F32, btf2_d)

            ident = consts.tile([128, 128], F16, name="ident")
            make_identity(nc, ident[:])
            negid = consts.tile([128, 128], F16, name="negid")
            nc.scalar.activation(out=negid[:], in_=ident[:], func=AF.Copy,
                                 scale=NEG_BIG)
            ones = consts.tile([128, 128], F16, name="ones")
            nc.gpsimd.memset(ones[:], 1.0)
            # idbig[k, f] = 1 iff f == k + 384 (shifted identity for diag-kill)
            idbig = consts.tile([128, 1024], F16, name="idbig")
            nc.gpsimd.memset(idbig[:], 0.0)
            nc.gpsimd.affine_select(
                out=idbig[:], in_=idbig[:],
                compare_op=mybir.AluOpType.not_equal, fill=1.0,
                base=384, pattern=[[-1, 1024]], channel_multiplier=1)
            epsb = consts.tile([128, 1], F32, name="epsb")
            nc.gpsimd.memset(epsb[:], 1e-12)

            # per-image tile state
            st = [{} for _ in range(IMGS_PER_CORE)]

            def T(img, tag, shape, dtype):
                t = pools[img].tile(shape, dtype, name=f"{tag}_{img}", tag=tag)
                st[img][tag] = t
                return t

            def phase_pre(img):
                """load x, g1 conv, feature norms."""
                xh = T(img, "xh", [128, 2, N], F16)
                for t in range(2):
                    nc.sync.dma_start(out=xh[:, t, :],
                                      in_=xh_d[img, t * 128:(t + 1) * 128, :])
                featT = T(img, "feat", [128, 2, N], F16)
                for to in range(2):
                    pss = [psums[img].tile([128, 512], F32,
                                           name=f"ps_g1_{img}_{to}_{nb}", tag="ps")
                           for nb in range(2)]
                    for kt in range(2):
                        for nb in range(2):
                            nc.tensor.matmul(
                                pss[nb][:], lhsT=wg1[:, kt, to * 128:(to + 1) * 128],
                                rhs=xh[:, kt, nb * 512:(nb + 1) * 512],
                                start=(kt == 0), stop=(kt == 1))
                    for nb in range(2):
                        nc.scalar.activation(
                            out=featT[:, to, nb * 512:(nb + 1) * 512], in_=pss[nb][:],
                            func=AF.Identity, bias=bt1[:, to:to + 1])
                fsq = T(img, "fx1", [128, 2, N], F16)
                for t in range(2):
                    nc.vector.tensor_mul(fsq[:, t, :], featT[:, t, :], featT[:, t, :])
                n2b = T(img, "n2b", [128, N], F32)
                for nb in range(2):
                    psn = psums[img].tile([128, 512], F32,
                                          name=f"ps_n2_{img}_{nb}", tag="ps")
                    for kt in range(2):
                        nc.tensor.matmul(
                            psn[:], lhsT=ones[:],
                            rhs=fsq[:, kt, nb * 512:(nb + 1) * 512],
                            start=(kt == 0), stop=(kt == 1))
                    nc.scalar.activation(out=n2b[:, nb * 512:(nb + 1) * 512],
                                         in_=psn[:], func=AF.Identity,
                                         bias=epsb[:, 0:1])
                rn2 = T(img, "fx1", [128, N], F32)  # reuses fsq slot (fsq dead)
                nc.vector.reciprocal_approx_fast(out=rn2[:], in_=n2b[:])
                invnb = T(img, "invnb", [128, N], F16)
                nc.scalar.activation(out=invnb[:], in_=rn2[:], func=AF.Sqrt)
                xnT = T(img, "n2b", [128, 2, N], F16)  # reuses n2b slot
                for t in range(2):
                    nc.vector.tensor_mul(xnT[:, t, :], featT[:, t, :], invnb[:])

            def phase_sel(img, Is):
                """sim I-blocks + top-8 selection for I in Is."""
                featT = st[img]["feat"]
                xnT = st[img]["n2b"]
                if 0 in Is:
                    st[img]["ixt"] = T(img, "ix", [128, 4, 8, 2], dt.uint16)
                ixbuf = st[img]["ixt"]
                for I in Is:
                    simblk = pools[img].tile([128, N], F16, name=f"sim{img}_{I}",
                                             tag="sim", bufs=2)
                    pss = [psums[img].tile([128, 512], F32,
                                           name=f"ps_sim_{img}_{I}_{cb}", tag="ps")
                           for cb in range(2)]
                    for kt in range(2):
                        for cb in range(2):
                            has_diag = (cb == I // 4)
                            nc.tensor.matmul(
                                pss[cb][:], lhsT=featT[:, kt, I * 128:(I + 1) * 128],
                                rhs=xnT[:, kt, cb * 512:(cb + 1) * 512],
                                start=(kt == 0),
                                stop=(kt == 1 and not has_diag))
                    for cb in range(2):
                        if cb == I // 4:
                            off = I * 128 - cb * 512
                            nc.tensor.matmul(pss[cb][:], lhsT=negid[:],
                                             rhs=idbig[:, 384 - off:896 - off],
                                             start=False, stop=True)
                        nc.scalar.activation(
                            out=simblk[:, cb * 512:(cb + 1) * 512], in_=pss[cb][:],
                            func=AF.Copy)
                    mx = pools[img].tile([128, 8], F16, name=f"mx{img}_{I}",
                                         tag="mx", bufs=2)
                    nc.vector.max(out=mx[:], in_=simblk[:])
                    nc.vector.max_index(out=ixbuf[:, I // 2, :, I % 2],
                                        in_max=mx[:], in_values=simblk[:])

            def phase_wrap(img):
                """wrapped idx buffer [128, 512] i16.
                col = 128*s + 16*k + 8*i + g ; idx number within s-block
                = 256*k + 128*i + 16*g + p16 -> node 256*s + 128*i + 16*g + p16
                """
                wrapped = T(img, "wrap", [128, 512], dt.int16)
                wview = wrapped[0:16, :].rearrange(
                    "p (s k i g) -> p s k i g", s=4, k=8, i=2, g=8)
                ixi = st[img]["ix"][:].bitcast(dt.int16)
                for g in range(8):
                    nc.sync.dma_start(out=wview[:, :, :, :, g],
                                      in_=ixi[16 * g:16 * (g + 1), :, :, :])
                nc.sync.dma_start(out=wrapped[16:32, :], in_=wrapped[0:16, :])
                nc.sync.dma_start(out=wrapped[32:64, :], in_=wrapped[0:32, :])
                nc.sync.dma_start(out=wrapped[64:128, :], in_=wrapped[0:64, :])

            def phase_qp(img):
                """q (n-major, to DRAM for gather) and p (n-major, +bp bias)."""
                featT = st[img]["feat"]
                q_sb = T(img, "q", [128, 8, 512], F16)
                q_dram = q_drams[img]
                for nt in range(8):
                    ps = psums[img].tile([128, 512], F32,
                                         name=f"ps_q_{img}_{nt}", tag="ps")
                    for kt in range(2):
                        nc.tensor.matmul(
                            ps[:], lhsT=featT[:, kt, nt * 128:(nt + 1) * 128],
                            rhs=wq[:, kt, :], start=(kt == 0), stop=(kt == 1))
                    nc.scalar.activation(out=q_sb[:, nt, :], in_=ps[:], func=AF.Copy)
                    nc.sync.dma_start(out=q_dram[nt * 128:(nt + 1) * 128, :],
                                      in_=q_sb[:, nt, :])
                p_nm = T(img, "pT", [128, 8, 512], F16)
                for nt in range(8):
                    ps = psums[img].tile([128, 512], F32,
                                         name=f"ps_p_{img}_{nt}", tag="ps")
                    for kt in range(2):
                        nc.tensor.matmul(
                            ps[:], lhsT=featT[:, kt, nt * 128:(nt + 1) * 128],
                            rhs=wp[:, kt, :], start=(kt == 0), stop=False)
                    # per-channel EdgeConv bias via K=1 broadcast matmul
                    nc.tensor.matmul(ps[:], lhsT=ones[0:1, :], rhs=bpv[0:1, :],
                                     start=False, stop=True)
                    nc.scalar.activation(out=p_nm[:, nt, :], in_=ps[:], func=AF.Copy)

            def phase_gather_dma(img, s):
                """issue the neighbor gather for s-block (n-major output)."""
                if s == 0:
                    st[img]["maxqt"] = T(img, "maxq", [128, 8, 512], F16)
                wrapped = st[img]["wrap"]
                go = pool_gath.tile([128, 16, 512], F16, name=f"go{img}_{s}",
                                    tag="go")
                st[img][f"go{s}"] = go
                nc.gpsimd.dma_gather(
                    out_ap=go[:], in_ap=q_drams[img][:],
                    idxs_ap=wrapped[:, 128 * s:128 * (s + 1)],
                    num_idxs=2048, num_idxs_reg=2048, elem_size=512,
                    transpose=False, single_packet=False)

            def phase_fold(img, s):
                """8-way max fold + self term for s-block."""
                maxq = st[img]["maxqt"]
                q_sb = st[img]["q"]
                go = st[img][f"go{s}"]
                gv = go[:].rearrange("p (k h) c -> p k h c", k=8)
                nc.vector.tensor_max(gv[:, 4:8, :, :], gv[:, 0:4, :, :],
                                     gv[:, 4:8, :, :])
                nc.vector.tensor_max(gv[:, 6:8, :, :], gv[:, 4:6, :, :],
                                     gv[:, 6:8, :, :])
                nc.vector.tensor_max(gv[:, 7, :, :], gv[:, 6, :, :],
                                     gv[:, 7, :, :])
                nc.vector.tensor_max(maxq[:, 2 * s:2 * s + 2, :],
                                     gv[:, 7, :, :], q_sb[:, 2 * s:2 * s + 2, :])

            def phase_edge(img):
                """e = relu(p + maxq) in n-major, then PE-transpose to
                ch-major eT [128, 4, N]."""
                p_nm = st[img]["pT"]
                maxq = st[img]["maxqt"]
                e_nm = T(img, "n2b", [128, 8, 512], F16)  # reuses xnT slot
                flat = [t[:].rearrange("p a c -> p (a c)")
                        for t in (e_nm, p_nm, maxq)]
                nc.vector.tensor_add(flat[0], flat[1], flat[2])
                nc.vector.tensor_scalar_max(flat[0], flat[0], 0.0)
                eT = T(img, "q", [128, 4, N], F16)  # reuses q slot (q dead)
                for a in range(4):
                    for nb in range(2):
                        ps = psums[img].tile([128, 512], F16,
                                             name=f"ps_tr_{img}_{a}_{nb}", tag="ps")
                        for j in range(4):
                            nc.tensor.transpose(
                                out=ps[:, j * 128:(j + 1) * 128],
                                in_=e_nm[:, 4 * nb + j, a * 128:(a + 1) * 128],
                                identity=ident[:])
                        nc.scalar.activation(
                            out=eT[:, a, nb * 512:(nb + 1) * 512], in_=ps[:],
                            func=AF.Copy)

            def phase_g2(img):
                """g2 conv + residual (ident@xh in PSUM) -> hc f16."""
                eT = st[img]["q"]
                xh = st[img]["xh"]
                hc = T(img, "hc", [128, 2, N], F16)
                for to in range(2):
                    pss = [psums[img].tile([128, 512], F32,
                                           name=f"ps_g2_{img}_{to}_{nb}", tag="ps")
                           for nb in range(2)]
                    for kt in range(4):
                        for nb in range(2):
                            nc.tensor.matmul(
                                pss[nb][:], lhsT=wg2[:, kt, to * 128:(to + 1) * 128],
                                rhs=eT[:, kt, nb * 512:(nb + 1) * 512],
                                start=(kt == 0), stop=False)
                    for nb in range(2):
                        nc.tensor.matmul(
                            pss[nb][:], lhsT=ident[:],
                            rhs=xh[:, to, nb * 512:(nb + 1) * 512],
                            start=False, stop=True)
                        nc.scalar.activation(
                            out=hc[:, to, nb * 512:(nb + 1) * 512], in_=pss[nb][:],
                            func=AF.Identity, bias=bt2[:, to:to + 1])

            def phase_f1(img):
                hc = st[img]["hc"]
                f1o = T(img, "f1o", [128, 8, N], F16)
                for to in range(8):
                    pss = [psums[img].tile([128, 512], F32,
                                           name=f"ps_f1_{img}_{to}_{nb}", tag="ps")
                           for nb in range(2)]
                    for kt in range(2):
                        for nb in range(2):
                            nc.tensor.matmul(
                                pss[nb][:], lhsT=wf1[:, kt, to * 128:(to + 1) * 128],
                                rhs=hc[:, kt, nb * 512:(nb + 1) * 512],
                                start=(kt == 0), stop=(kt == 1))
                    for nb in range(2):
                        nc.scalar.activation(
                            out=f1o[:, to, nb * 512:(nb + 1) * 512], in_=pss[nb][:],
                            func=AF.Relu, bias=bbf1[:, to:to + 1])

            def phase_f2(img):
                f1o = st[img]["f1o"]
                hc = st[img]["hc"]
                h2c = T(img, "pT", [128, 2, N], F16)  # reuses p slot (p dead)
                for to in range(2):
                    pss = [psums[img].tile([128, 512], F32,
                                           name=f"ps_f2_{img}_{to}_{nb}", tag="ps")
                           for nb in range(2)]
                    for kt in range(8):
                        for nb in range(2):
                            nc.tensor.matmul(
                                pss[nb][:], lhsT=wf2[:, kt, to * 128:(to + 1) * 128],
                                rhs=f1o[:, kt, nb * 512:(nb + 1) * 512],
                                start=(kt == 0), stop=False)
                    for nb in range(2):
                        nc.tensor.matmul(
                            pss[nb][:], lhsT=ident[:],
                            rhs=hc[:, to, nb * 512:(nb + 1) * 512],
                            start=False, stop=True)
                        nc.scalar.activation(
                            out=h2c[:, to, nb * 512:(nb + 1) * 512], in_=pss[nb][:],
                            func=AF.Identity, bias=bbf2[:, to:to + 1])

            def phase_bneck(img):
                h2c = st[img]["pT"]
                pad = T(img, "pad", [64, 34 * 34], F16)
                nc.vector.memset(pad[:], 0.0)
                pad3 = pad[:].rearrange("p (r c) -> p r c", r=34)
                for nb in range(2):
                    ps = psum64s[img].tile([64, 512], F32,
                                           name=f"ps_b1_{img}_{nb}", tag="ps64")
                    for kt in range(2):
                        nc.tensor.matmul(
                            ps[:], lhsT=wb1[:, kt, :],
                            rhs=h2c[:, kt, nb * 512:(nb + 1) * 512],
                            start=(kt == 0), stop=(kt == 1))
                    # evacuate straight into the zero-padded conv input
                    nc.scalar.activation(
                        out=pad3[:, 1 + 16 * nb:17 + 16 * nb, 1:33],
                        in_=ps[:].rearrange("p (r c) -> p r c", r=16),
                        func=AF.Relu, bias=btb1[:, 0:1])
                b2o = T(img, "b2o", [64, N], F16)
                for nb in range(2):
                    ps = psum64s[img].tile([64, 512], F32,
                                           name=f"ps_b2_{img}_{nb}", tag="ps64")
                    for tap in range(9):
                        dy, dx = tap // 3, tap % 3
                        rhs = pad3[:, 16 * nb + dy:16 * nb + dy + 16, dx:dx + 32]
                        nc.tensor.matmul(ps[:], lhsT=wb2[:, tap, :], rhs=rhs,
                                         start=(tap == 0), stop=(tap == 8))
                    nc.scalar.activation(out=b2o[:, nb * 512:(nb + 1) * 512],
                                         in_=ps[:], func=AF.Relu, bias=btb2[:, 0:1])

            def phase_out(img):
                """b3 + h2 + x residuals in PSUM; final BN in the evacuation."""
                h2c = st[img]["pT"]
                xh = st[img]["xh"]
                b2o = st[img]["b2o"]
                out32 = T(img, "maxq", [128, 2, N], F32)  # reuses maxq slot
                for to in range(2):
                    pss = [psums[img].tile([128, 512], F32,
                                           name=f"ps_b3_{img}_{to}_{nb}", tag="ps")
                           for nb in range(2)]
                    for nb in range(2):
                        sl = slice(nb * 512, (nb + 1) * 512)
                        nc.tensor.matmul(
                            pss[nb][:], lhsT=wb3[:, to * 128:(to + 1) * 128],
                            rhs=b2o[:, sl], start=True, stop=False)
                        nc.tensor.matmul(
                            pss[nb][:], lhsT=ident[:], rhs=h2c[:, to, sl],
                            start=False, stop=False)
                        nc.tensor.matmul(
                            pss[nb][:], lhsT=ident[:], rhs=xh[:, to, sl],
                            start=False, stop=True)
                        nc.scalar.activation(
                            out=out32[:, to, sl], in_=pss[nb][:],
                            func=AF.Identity, scale=bsf[:, to:to + 1],
                            bias=btf2[:, to:to + 1])
                    for nb in range(2):
                        nc.sync.dma_start(
                            out=out_d[img, to * 128:(to + 1) * 128,
                                      nb * 512:(nb + 1) * 512],
                            in_=out32[:, to, nb * 512:(nb + 1) * 512])

            # ---- two-image software pipeline (emission order = priority;
            # engine queues execute in order, so image 1's DVE/PE work must be
            # emitted BEFORE image 0's folds or it stalls behind them) ----
            phase_pre(0)
            phase_sel(0, range(8))
            phase_wrap(0)
            phase_qp(0)
            phase_pre(1)
            phase_gather_dma(0, 0)
            phase_fold(0, 0)
            phase_sel(1, range(0, 4))
            phase_gather_dma(0, 1)
            phase_fold(0, 1)
            phase_sel(1, range(4, 8))
            phase_gather_dma(0, 2)
            phase_fold(0, 2)
            phase_wrap(1)
            phase_qp(1)
            phase_gather_dma(0, 3)
            phase_fold(0, 3)
            phase_edge(0)
            phase_g2(0)
            phase_gather_dma(1, 0)
            phase_fold(1, 0)
            phase_f1(0)
            phase_gather_dma(1, 1)
            phase_fold(1, 1)
            phase_f2(0)
            phase_gather_dma(1, 2)
            phase_fold(1, 2)
            phase_bneck(0)
            phase_gather_dma(1, 3)
            phase_fold(1, 3)
            phase_out(0)
            phase_edge(1)
            phase_g2(1)
            phase_f1(1)
            phase_f2(1)
            phase_bneck(1)
            phase_out(1)

    nc.finalize()
    return nc


# --------------------------------------------------------------------------
# entry point
# --------------------------------------------------------------------------
def kernel(**inputs):
    inp = {k: np.asarray(v) for k, v in inputs.items()}
    w = _prep_weights(inp)

    if 'nc' not in _cache:
        _cache['nc'] = _build_bass()
    nc = _cache['nc']

    xh = inp['x'].astype(np.float16).reshape(B, C, N)
    in_maps = []
    for c in range(N_CORES):
        m = {'xh': np.ascontiguousarray(xh[c * 2:(c + 1) * 2])}
        m.update(w)
        in_maps.append(m)

    from concourse.bass_utils import run_bass_kernel_spmd
    trace = bool(os.environ.get("KBENCH_TRACE"))
    res = run_bass_kernel_spmd(nc, in_maps, core_ids=list(range(N_CORES)),
                               trace=trace)
    _cache['exec_time_ns'] = res.exec_time_ns
    _cache['results'] = res
    out = np.zeros((B, C, N), np.float32)
    for c in range(N_CORES):
        out[c * 2:(c + 1) * 2] = res.results[c]['out']
    return out.reshape(B, C, H, W)


# revision 13
# speedup vs baseline: 1.8415x; 1.1497x over previous
"""Trainium2 Bass kernel for nn_Block_17033840296551 (GNN message passing block).

Data-parallel over batch: 16 images -> 8 cores x 2 images. Each core runs the
full block (g1 conv -> kNN top-8+self -> EdgeConv max-agg -> g2 -> FFN ->
bottleneck -> final BN) on its 2 images with no cross-core communication.

v3 (gather + overlap rewrite):
  * NON-transposed dma_gather (n-major output): the transposed gather's rx
    side emits one 256B xbar-spray descriptor per 256 payload bytes (4x the
    descriptors), which made each 2k-idx gather ~20us of gpsimd descgen.
    n-major gathers are ~2x cheaper on the Q7s. The EdgeConv max-fold runs
    in n-major layout (where the self term is q_sb itself, no qT needed);
    e = relu(p+maxq) is then transposed to ch-major via 32 PE tile
    transposes for the g2 matmul.
  * Two-image software pipeline by construction: per-image tile pools AND
    per-image PSUM pools (a shared psum pool's slot rotation chained image
    1's first matmul behind image 0's last evacuation), with emission
    interleaved so image 1's compute phases sit inside image 0's gather
    window and vice versa.
  * All residual adds (g2, f2, b3, outer) ride the PSUM accumulation as
    identity matmuls; final BN fused into the last evacuation (scale/bias).
  * EdgeConv bias bp enters as a K=1 matmul (ones row x bp row) since it is
    per-channel and channels sit on the free axis in n-major layout.
  * Norms: n2 broadcast to 128 partitions via all-ones matmul, rsqrt via
    reciprocal_approx_fast + Sqrt activation.
"""

import os
import numpy as np

# problem constants (hardcoded per harness contract)
B, C, H, W = 16, 256, 32, 32
N = H * W           # 1024 pixels per image
K = 9
EPS = 1e-5
IMGS_PER_CORE = 2
N_CORES = 8
NEG_BIG = -30000.0

_cache = {}


# --------------------------------------------------------------------------
# host-side preprocessing
# --------------------------------------------------------------------------
def _bn_fold(p):
    g, b, m, v = np.asarray(p, np.float32)
    s = g / np.sqrt(v + EPS)
    t = b - m * s
    return s, t


def _pack_kxm(w_t, part=128):
    """[K, M] -> [part, K//part, M] (partition-major K tiling)."""
    Kd, M = w_t.shape
    kt = Kd // part
    return np.ascontiguousarray(w_t.reshape(kt, part, M).transpose(1, 0, 2))


def _pack_bias(b, part=128):
    n = b.shape[0]
    t = n // part
    return np.ascontiguousarray(b.reshape(t, part).T)  # [part, t]


def _prep_weights(inp):
    f16 = np.float16
    s1, t1 = _bn_fold(inp['g1_bn'])
    Wg1 = s1[:, None] * inp['g1_w']
    s2, t2 = _bn_fold(inp['g2_bn'])
    Wg2 = s2[:, None] * inp['g2_w']
    sf1, tf1 = _bn_fold(inp['f1_bn'])
    Wf1 = sf1[:, None] * inp['f1_w']
    bf1 = sf1 * inp['f1_b'] + tf1
    sf2, tf2 = _bn_fold(inp['f2_bn'])
    Wf2 = sf2[:, None] * inp['f2_w']
    bf2 = sf2 * inp['f2_b'] + tf2
    sb1, tb1 = _bn_fold(inp['b1_bn'])
    Wb1 = sb1[:, None] * inp['b1_w']
    sb2, tb2 = _bn_fold(inp['b2_bn'])
    Wb2 = sb2[:, None, None, None] * inp['b2_w']
    sb3, tb3 = _bn_fold(inp['b3_bn'])
    Wb3 = sb3[:, None] * inp['b3_w']
    sf, tf = _bn_fold(inp['bnf'])
    # final BN absorbs b3's bias: out = sf*(P + tb3) + tf, P = b3conv+h2+x
    btf2 = sf * tb3 + tf

    A = inp['edge_w'][:, :C]
    Bm = inp['edge_w'][:, C:]
    Wp = A - Bm
    Wq = Bm
    bp = inp['edge_b']

    wb2_t = np.zeros((64, 9, 64), f16)
    for dy in range(3):
        for dx in range(3):
            wb2_t[:, dy * 3 + dx, :] = Wb2[:, :, dy, dx].T.astype(f16)

    return {
        'wg1': _pack_kxm(Wg1.T.astype(f16)),           # [128,2,256]
        'wp': _pack_kxm(Wp.T.astype(f16)),             # [128,2,512]
        'wq': _pack_kxm(Wq.T.astype(f16)),             # [128,2,512]
        'wg2': _pack_kxm(Wg2.T.astype(f16)),           # [128,4,256]
        'wf1': _pack_kxm(Wf1.T.astype(f16)),           # [128,2,1024]
        'wf2': _pack_kxm(Wf2.T.astype(f16)),           # [128,8,256]
        'wb1': _pack_kxm(Wb1.T.astype(f16)),           # [128,2,64]
        'wb2': wb2_t,                                   # [64,9,64]
        'wb3': Wb3.T.astype(f16),                       # [64,256]
        'bt1': _pack_bias(t1),                          # [128,2] f32
        'bt2': _pack_bias(t2),
        'bpv': np.ascontiguousarray(bp.astype(f16).reshape(1, 512)),
        'bbf1': _pack_bias(bf1),                        # [128,8]
        'bbf2': _pack_bias(bf2),
        'btb1': np.ascontiguousarray(tb1[:, None].astype(np.float32)),  # [64,1]
        'btb2': np.ascontiguousarray(tb2[:, None].astype(np.float32)),
        'bsf': _pack_bias(sf),
        'btf2': _pack_bias(btf2),
    }


# --------------------------------------------------------------------------
# device kernel builder
# --------------------------------------------------------------------------
def _build_bass():
    import concourse.bass as bass
    import concourse.mybir as mybir
    from concourse import bacc
    from concourse.tile import TileContext
    from concourse.masks import make_identity

    dt = mybir.dt
    F16 = dt.float16
    F32 = dt.float32
    AF = mybir.ActivationFunctionType

    nc = bacc.Bacc()

    # ---- DRAM parameters ----
    xh_d = nc.declare_dram_parameter("xh", [IMGS_PER_CORE, C, N], F16, isOutput=False)
    wg1_d = nc.declare_dram_parameter("wg1", [128, 2, 256], F16, isOutput=False)
    wp_d = nc.declare_dram_parameter("wp", [128, 2, 512], F16, isOutput=False)
    wq_d = nc.declare_dram_parameter("wq", [128, 2, 512], F16, isOutput=False)
    wg2_d = nc.declare_dram_parameter("wg2", [128, 4, 256], F16, isOutput=False)
    wf1_d = nc.declare_dram_parameter("wf1", [128, 2, 1024], F16, isOutput=False)
    wf2_d = nc.declare_dram_parameter("wf2", [128, 8, 256], F16, isOutput=False)
    wb1_d = nc.declare_dram_parameter("wb1", [128, 2, 64], F16, isOutput=False)
    wb2_d = nc.declare_dram_parameter("wb2", [64, 9, 64], F16, isOutput=False)
    wb3_d = nc.declare_dram_parameter("wb3", [64, 256], F16, isOutput=False)
    bt1_d = nc.declare_dram_parameter("bt1", [128, 2], F32, isOutput=False)
    bt2_d = nc.declare_dram_parameter("bt2", [128, 2], F32, isOutput=False)
    bpv_d = nc.declare_dram_parameter("bpv", [1, 512], F16, isOutput=False)
    bbf1_d = nc.declare_dram_parameter("bbf1", [128, 8], F32, isOutput=False)
    bbf2_d = nc.declare_dram_parameter("bbf2", [128, 2], F32, isOutput=False)
    btb1_d = nc.declare_dram_parameter("btb1", [64, 1], F32, isOutput=False)
    btb2_d = nc.declare_dram_parameter("btb2", [64, 1], F32, isOutput=False)
    bsf_d = nc.declare_dram_parameter("bsf", [128, 2], F32, isOutput=False)
    btf2_d = nc.declare_dram_parameter("btf2", [128, 2], F32, isOutput=False)
    q_drams = [nc.dram_tensor(f"q_dram{i}", [N, 512], F16)
               for i in range(IMGS_PER_CORE)]
    out_d = nc.declare_dram_parameter("out", [IMGS_PER_CORE, C, N], F32, isOutput=True)

    with TileContext(nc) as tc:
        import contextlib
        ctx = contextlib.ExitStack()
        with ctx:
            consts = ctx.enter_context(tc.tile_pool(name="consts", bufs=1))
            # per-image pools: every tag is a dedicated buffer; PSUM pools are
            # per-image so slot rotation never chains one image behind the other
            pools = [ctx.enter_context(tc.tile_pool(name=f"im{i}", bufs=1))
                     for i in range(IMGS_PER_CORE)]
            psums = [ctx.enter_context(
                tc.tile_pool(name=f"psum{i}", bufs=3, space="PSUM"))
                for i in range(IMGS_PER_CORE)]
            psum64s = [ctx.enter_context(
                tc.tile_pool(name=f"psum64_{i}", bufs=1, space="PSUM"))
                for i in range(IMGS_PER_CORE)]
            pool_gath = ctx.enter_context(tc.tile_pool(name="gath", bufs=2))

            # ---- constants / weights (loaded once) ----
            # alternate sync/scalar HWDGE queues so ~20 loads don't serialize
            # in front of the first matmul
            _ldq = [0]

            def load(name, shape, dtype, src, eng=None):
                t = consts.tile(shape, dtype, name=name)
                if eng is None:
                    eng = nc.sync if _ldq[0] % 2 == 0 else nc.scalar
                    _ldq[0] += 1
                eng.dma_start(out=t[:], in_=src[:])
                return t

            wg1 = load("wg1s", [128, 2, 256], F16, wg1_d, eng=nc.sync)
            wp = load("wps", [128, 2, 512], F16, wp_d)
            wq = load("wqs", [128, 2, 512], F16, wq_d)
            wg2 = load("wg2s", [128, 4, 256], F16, wg2_d)
            wf1 = load("wf1s", [128, 2, 1024], F16, wf1_d)
            wf2 = load("wf2s", [128, 8, 256], F16, wf2_d)
            wb1 = load("wb1s", [128, 2, 64], F16, wb1_d)
            wb2 = load("wb2s", [64, 9, 64], F16, wb2_d)
            wb3 = load("wb3s", [64, 256], F16, wb3_d)
            bt1 = load("bt1s", [128, 2], F32, bt1_d, eng=nc.scalar)
            bt2 = load("bt2s", [128, 2], F32, bt2_d)
            bpv = load("bpvs", [1, 512], F16, bpv_d)
            bbf1 = load("bbf1s", [128, 8], F32, bbf1_d)
            bbf2 = load("bbf2s", [128, 2], F32, bbf2_d)
            btb1 = load("btb1s", [64, 1], F32, btb1_d)
            btb2 = load("btb2s", [64, 1], F32, btb2_d)
            bsf = load("bsfs", [128, 2], F32, bsf_d)
            btf2 = load("btf2s", [128, 2], F32, btf2_d)

            ident = consts.tile([128, 128], F16, name="ident")
            make_identity(nc, ident[:])
            negid = consts.tile([128, 128], F16, name="negid")
            nc.scalar.activation(out=negid[:], in_=ident[:], func=AF.Copy,
                                 scale=NEG_BIG)
            ones = consts.tile([128, 128], F16, name="ones")
            nc.gpsimd.memset(ones[:], 1.0)
            # idbig[k, f] = 1 iff f == k + 384 (shifted identity for diag-kill)
            idbig = consts.tile([128, 1024], F16, name="idbig")
            nc.gpsimd.memset(idbig[:], 0.0)
            nc.gpsimd.affine_select(
                out=idbig[:], in_=idbig[:],
                compare_op=mybir.AluOpType.not_equal, fill=1.0,
                base=384, pattern=[[-1, 1024]], channel_multiplier=1)
            epsb = consts.tile([128, 1], F32, name="epsb")
            nc.gpsimd.memset(epsb[:], 1e-12)
            # dummy gather: loads the gather ucode into Q7 IRAM now instead of
            # in front of image 0's first real gather
            zidx = consts.tile([128, 8], dt.int16, name="zidx")
            nc.gpsimd.memset(zidx[:], 0)
            scrg = consts.tile([128, 1, 512], F16, name="scrg")
            nc.gpsimd.dma_gather(
                out_ap=scrg[:], in_ap=q_drams[0][:], idxs_ap=zidx[:],
                num_idxs=128, num_idxs_reg=128, elem_size=512,
                transpose=False, single_packet=False)

            # per-image tile state
            st = [{} for _ in range(IMGS_PER_CORE)]

            def T(img, tag, shape, dtype):
                t = pools[img].tile(shape, dtype, name=f"{tag}_{img}", tag=tag)
                st[img][tag] = t
                return t

            def phase_pre(img):
                """load x, g1 conv, feature norms."""
                xh = T(img, "xh", [128, 2, N], F16)
                for t in range(2):
                    nc.sync.dma_start(out=xh[:, t, :],
                                      in_=xh_d[img, t * 128:(t + 1) * 128, :])
                featT = T(img, "feat", [128, 2, N], F16)
                for to in range(2):
                    pss = [psums[img].tile([128, 512], F32,
                                           name=f"ps_g1_{img}_{to}_{nb}", tag="ps")
                           for nb in range(2)]
                    for kt in range(2):
                        for nb in range(2):
                            nc.tensor.matmul(
                                pss[nb][:], lhsT=wg1[:, kt, to * 128:(to + 1) * 128],
                                rhs=xh[:, kt, nb * 512:(nb + 1) * 512],
                                start=(kt == 0), stop=(kt == 1))
                    for nb in range(2):
                        nc.scalar.activation(
                            out=featT[:, to, nb * 512:(nb + 1) * 512], in_=pss[nb][:],
                            func=AF.Identity, bias=bt1[:, to:to + 1])
                fsq = T(img, "fx1", [128, 2, N], F16)
                for t in range(2):
                    nc.vector.tensor_mul(fsq[:, t, :], featT[:, t, :], featT[:, t, :])
                n2b = T(img, "n2b", [128, N], F32)
                for nb in range(2):
                    psn = psums[img].tile([128, 512], F32,
                                          name=f"ps_n2_{img}_{nb}", tag="ps")
                    for kt in range(2):
                        nc.tensor.matmul(
                            psn[:], lhsT=ones[:],
                            rhs=fsq[:, kt, nb * 512:(nb + 1) * 512],
                            start=(kt == 0), stop=(kt == 1))
                    nc.scalar.activation(out=n2b[:, nb * 512:(nb + 1) * 512],
                                         in_=psn[:], func=AF.Identity,
                                         bias=epsb[:, 0:1])
                rn2 = T(img, "fx1", [128, N], F32)  # reuses fsq slot (fsq dead)
                nc.vector.reciprocal_approx_fast(out=rn2[:], in_=n2b[:])
                invnb = T(img, "invnb", [128, N], F16)
                nc.scalar.activation(out=invnb[:], in_=rn2[:], func=AF.Sqrt)
                xnT = T(img, "n2b", [128, 2, N], F16)  # reuses n2b slot
                for t in range(2):
                    nc.vector.tensor_mul(xnT[:, t, :], featT[:, t, :], invnb[:])

            def phase_sel(img, Is):
                """sim I-blocks + top-8 selection for I in Is."""
                featT = st[img]["feat"]
                xnT = st[img]["n2b"]
                if 0 in Is:
                    st[img]["ixt"] = T(img, "ix", [128, 4, 8, 2], dt.uint16)
                ixbuf = st[img]["ixt"]
                for I in Is:
                    simblk = pools[img].tile([128, N], F16, name=f"sim{img}_{I}",
                                             tag="sim", bufs=2)
                    pss = [psums[img].tile([128, 512], F32,
                                           name=f"ps_sim_{img}_{I}_{cb}", tag="ps")
                           for cb in range(2)]
                    for kt in range(2):
                        for cb in range(2):
                            has_diag = (cb == I // 4)
                            nc.tensor.matmul(
                                pss[cb][:], lhsT=featT[:, kt, I * 128:(I + 1) * 128],
                                rhs=xnT[:, kt, cb * 512:(cb + 1) * 512],
                                start=(kt == 0),
                                stop=(kt == 1 and not has_diag))
                    for cb in range(2):
                        if cb == I // 4:
                            off = I * 128 - cb * 512
                            nc.tensor.matmul(pss[cb][:], lhsT=negid[:],
                                             rhs=idbig[:, 384 - off:896 - off],
                                             start=False, stop=True)
                        nc.scalar.activation(
                            out=simblk[:, cb * 512:(cb + 1) * 512], in_=pss[cb][:],
                            func=AF.Copy)
                    mx = pools[img].tile([128, 8], F16, name=f"mx{img}_{I}",
                                         tag="mx", bufs=2)
                    nc.vector.max(out=mx[:], in_=simblk[:])
                    nc.vector.max_index(out=ixbuf[:, I // 2, :, I % 2],
                                        in_max=mx[:], in_values=simblk[:])

            def phase_wrap(img):
                """wrapped idx buffer [128, 512] i16.
                col = 128*s + 16*k + 8*i + g ; idx number within s-block
                = 256*k + 128*i + 16*g + p16 -> node 256*s + 128*i + 16*g + p16
                """
                wrapped = T(img, "wrap", [128, 512], dt.int16)
                wtmp = T(img, "wtmp", [16, 8, 64], dt.int16)
                ixi = st[img]["ix"][:].bitcast(dt.int16)
                ixf = ixi.rearrange("p s k i -> p (s k i)")
                for g in range(8):
                    nc.sync.dma_start(out=wtmp[:, g, :],
                                      in_=ixf[16 * g:16 * (g + 1), :])
                # per-partition (g,s,k,i) -> (s,k,i,g) permute on DVE
                nc.vector.tensor_copy(
                    wrapped[0:16, :].rearrange(
                        "p (s k i g) -> p s k i g", s=4, k=8, i=2, g=8),
                    wtmp[:].rearrange("p g (s k i) -> p s k i g", s=4, k=8, i=2))
                nc.sync.dma_start(out=wrapped[16:32, :], in_=wrapped[0:16, :])
                nc.sync.dma_start(out=wrapped[32:64, :], in_=wrapped[0:32, :])
                nc.sync.dma_start(out=wrapped[64:128, :], in_=wrapped[0:64, :])

            def phase_qp(img):
                """q (n-major, to DRAM for gather) and p (n-major, +bp bias)."""
                featT = st[img]["feat"]
                q_sb = T(img, "q", [128, 8, 512], F16)
                q_dram = q_drams[img]
                for nt in range(8):
                    ps = psums[img].tile([128, 512], F32,
                                         name=f"ps_q_{img}_{nt}", tag="ps")
                    for kt in range(2):
                        nc.tensor.matmul(
                            ps[:], lhsT=featT[:, kt, nt * 128:(nt + 1) * 128],
                            rhs=wq[:, kt, :], start=(kt == 0), stop=(kt == 1))
                    nc.scalar.activation(out=q_sb[:, nt, :], in_=ps[:], func=AF.Copy)
                    nc.sync.dma_start(out=q_dram[nt * 128:(nt + 1) * 128, :],
                                      in_=q_sb[:, nt, :])
                p_nm = T(img, "pT", [128, 8, 512], F16)
                for nt in range(8):
                    ps = psums[img].tile([128, 512], F32,
                                         name=f"ps_p_{img}_{nt}", tag="ps")
                    for kt in range(2):
                        nc.tensor.matmul(
                            ps[:], lhsT=featT[:, kt, nt * 128:(nt + 1) * 128],
                            rhs=wp[:, kt, :], start=(kt == 0), stop=False)
                    # per-channel EdgeConv bias via K=1 broadcast matmul
                    nc.tensor.matmul(ps[:], lhsT=ones[0:1, :], rhs=bpv[0:1, :],
                                     start=False, stop=True)
                    nc.scalar.activation(out=p_nm[:, nt, :], in_=ps[:], func=AF.Copy)

            def phase_gather_dma(img, s):
                """issue the neighbor gather for s-block (n-major output)."""
                if s == 0:
                    st[img]["maxqt"] = T(img, "maxq", [128, 8, 512], F16)
                wrapped = st[img]["wrap"]
                go = pool_gath.tile([128, 16, 512], F16, name=f"go{img}_{s}",
                                    tag="go")
                st[img][f"go{s}"] = go
                nc.gpsimd.dma_gather(
                    out_ap=go[:], in_ap=q_drams[img][:],
                    idxs_ap=wrapped[:, 128 * s:128 * (s + 1)],
                    num_idxs=2048, num_idxs_reg=2048, elem_size=512,
                    transpose=False, single_packet=False)

            def phase_fold(img, s):
                """8-way max fold + self term for s-block."""
                maxq = st[img]["maxqt"]
                q_sb = st[img]["q"]
                go = st[img][f"go{s}"]
                gv = go[:].rearrange("p (k h) c -> p k h c", k=8)
                nc.vector.tensor_max(gv[:, 4:8, :, :], gv[:, 0:4, :, :],
                                     gv[:, 4:8, :, :])
                nc.vector.tensor_max(gv[:, 6:8, :, :], gv[:, 4:6, :, :],
                                     gv[:, 6:8, :, :])
                nc.vector.tensor_max(gv[:, 7, :, :], gv[:, 6, :, :],
                                     gv[:, 7, :, :])
                nc.vector.tensor_max(maxq[:, 2 * s:2 * s + 2, :],
                                     gv[:, 7, :, :], q_sb[:, 2 * s:2 * s + 2, :])

            def phase_edge(img):
                """e = relu(p + maxq) in n-major, then PE-transpose to
                ch-major eT [128, 4, N]."""
                p_nm = st[img]["pT"]
                maxq = st[img]["maxqt"]
                e_nm = T(img, "n2b", [128, 8, 512], F16)  # reuses xnT slot
                flat = [t[:].rearrange("p a c -> p (a c)")
                        for t in (e_nm, p_nm, maxq)]
                nc.vector.tensor_add(flat[0], flat[1], flat[2])
                nc.vector.tensor_scalar_max(flat[0], flat[0], 0.0)
                eT = T(img, "q", [128, 4, N], F16)  # reuses q slot (q dead)
                for a in range(4):
                    for nb in range(2):
                        ps = psums[img].tile([128, 512], F16,
                                             name=f"ps_tr_{img}_{a}_{nb}", tag="ps")
                        for j in range(4):
                            nc.tensor.transpose(
                                out=ps[:, j * 128:(j + 1) * 128],
                                in_=e_nm[:, 4 * nb + j, a * 128:(a + 1) * 128],
                                identity=ident[:])
                        nc.scalar.activation(
                            out=eT[:, a, nb * 512:(nb + 1) * 512], in_=ps[:],
                            func=AF.Copy)

            def phase_g2(img):
                """g2 conv + residual (ident@xh in PSUM) -> hc f16."""
                eT = st[img]["q"]
                xh = st[img]["xh"]
                hc = T(img, "hc", [128, 2, N], F16)
                for to in range(2):
                    pss = [psums[img].tile([128, 512], F32,
                                           name=f"ps_g2_{img}_{to}_{nb}", tag="ps")
                           for nb in range(2)]
                    for kt in range(4):
                        for nb in range(2):
                            nc.tensor.matmul(
                                pss[nb][:], lhsT=wg2[:, kt, to * 128:(to + 1) * 128],
                                rhs=eT[:, kt, nb * 512:(nb + 1) * 512],
                                start=(kt == 0), stop=False)
                    for nb in range(2):
                        nc.tensor.matmul(
                            pss[nb][:], lhsT=ident[:],
                            rhs=xh[:, to, nb * 512:(nb + 1) * 512],
                            start=False, stop=True)
                        nc.scalar.activation(
                            out=hc[:, to, nb * 512:(nb + 1) * 512], in_=pss[nb][:],
                            func=AF.Identity, bias=bt2[:, to:to + 1])

            def phase_f1(img, dve_evac=False):
                hc = st[img]["hc"]
                f1o = T(img, "f1o", [128, 8, N], F16)
                for to in range(8):
                    pss = [psums[img].tile([128, 512], F32,
                                           name=f"ps_f1_{img}_{to}_{nb}", tag="ps")
                           for nb in range(2)]
                    for kt in range(2):
                        for nb in range(2):
                            nc.tensor.matmul(
                                pss[nb][:], lhsT=wf1[:, kt, to * 128:(to + 1) * 128],
                                rhs=hc[:, kt, nb * 512:(nb + 1) * 512],
                                start=(kt == 0), stop=(kt == 1))
                    for nb in range(2):
                        dst = f1o[:, to, nb * 512:(nb + 1) * 512]
                        if dve_evac and to % 2 == 1:
                            nc.vector.tensor_scalar(
                                out=dst, in0=pss[nb][:],
                                scalar1=bbf1[:, to:to + 1], scalar2=0.0,
                                op0=mybir.AluOpType.add,
                                op1=mybir.AluOpType.max)
                        else:
                            nc.scalar.activation(
                                out=dst, in_=pss[nb][:],
                                func=AF.Relu, bias=bbf1[:, to:to + 1])

            def phase_f2(img):
                f1o = st[img]["f1o"]
                hc = st[img]["hc"]
                h2c = T(img, "pT", [128, 2, N], F16)  # reuses p slot (p dead)
                for to in range(2):
                    pss = [psums[img].tile([128, 512], F32,
                                           name=f"ps_f2_{img}_{to}_{nb}", tag="ps")
                           for nb in range(2)]
                    for kt in range(8):
                        for nb in range(2):
                            nc.tensor.matmul(
                                pss[nb][:], lhsT=wf2[:, kt, to * 128:(to + 1) * 128],
                                rhs=f1o[:, kt, nb * 512:(nb + 1) * 512],
                                start=(kt == 0), stop=False)
                    for nb in range(2):
                        nc.tensor.matmul(
                            pss[nb][:], lhsT=ident[:],
                            rhs=hc[:, to, nb * 512:(nb + 1) * 512],
                            start=False, stop=True)
                        nc.scalar.activation(
                            out=h2c[:, to, nb * 512:(nb + 1) * 512], in_=pss[nb][:],
                            func=AF.Identity, bias=bbf2[:, to:to + 1])

            def phase_bneck(img):
                h2c = st[img]["pT"]
                pad = T(img, "pad", [64, 34 * 34], F16)
                nc.vector.memset(pad[:], 0.0)
                pad3 = pad[:].rearrange("p (r c) -> p r c", r=34)
                for nb in range(2):
                    ps = psum64s[img].tile([64, 512], F32,
                                           name=f"ps_b1_{img}_{nb}", tag="ps64")
                    for kt in range(2):
                        nc.tensor.matmul(
                            ps[:], lhsT=wb1[:, kt, :],
                            rhs=h2c[:, kt, nb * 512:(nb + 1) * 512],
                            start=(kt == 0), stop=(kt == 1))
                    # evacuate straight into the zero-padded conv input
                    nc.scalar.activation(
                        out=pad3[:, 1 + 16 * nb:17 + 16 * nb, 1:33],
                        in_=ps[:].rearrange("p (r c) -> p r c", r=16),
                        func=AF.Relu, bias=btb1[:, 0:1])
                b2o = T(img, "b2o", [64, N], F16)
                for nb in range(2):
                    ps = psum64s[img].tile([64, 512], F32,
                                           name=f"ps_b2_{img}_{nb}", tag="ps64")
                    for tap in range(9):
                        dy, dx = tap // 3, tap % 3
                        rhs = pad3[:, 16 * nb + dy:16 * nb + dy + 16, dx:dx + 32]
                        nc.tensor.matmul(ps[:], lhsT=wb2[:, tap, :], rhs=rhs,
                                         start=(tap == 0), stop=(tap == 8))
                    nc.scalar.activation(out=b2o[:, nb * 512:(nb + 1) * 512],
                                         in_=ps[:], func=AF.Relu, bias=btb2[:, 0:1])

            def phase_out(img):
                """b3 + h2 + x residuals in PSUM; final BN in the evacuation."""
                h2c = st[img]["pT"]
                xh = st[img]["xh"]
                b2o = st[img]["b2o"]
                out32 = T(img, "maxq", [128, 2, N], F32)  # reuses maxq slot
                for to in range(2):
                    pss = [psums[img].tile([128, 512], F32,
                                           name=f"ps_b3_{img}_{to}_{nb}", tag="ps")
                           for nb in range(2)]
                    for nb in range(2):
                        sl = slice(nb * 512, (nb + 1) * 512)
                        nc.tensor.matmul(
                            pss[nb][:], lhsT=wb3[:, to * 128:(to + 1) * 128],
                            rhs=b2o[:, sl], start=True, stop=False)
                        nc.tensor.matmul(
                            pss[nb][:], lhsT=ident[:], rhs=h2c[:, to, sl],
                            start=False, stop=False)
                        nc.tensor.matmul(
                            pss[nb][:], lhsT=ident[:], rhs=xh[:, to, sl],
                            start=False, stop=True)
                        nc.scalar.activation(
                            out=out32[:, to, sl], in_=pss[nb][:],
                            func=AF.Identity, scale=bsf[:, to:to + 1],
                            bias=btf2[:, to:to + 1])
                    for nb in range(2):
                        nc.sync.dma_start(
                            out=out_d[img, to * 128:(to + 1) * 128,
                                      nb * 512:(nb + 1) * 512],
                            in_=out32[:, to, nb * 512:(nb + 1) * 512])

            # ---- two-image software pipeline (emission order = priority;
            # engine queues execute in order, so image 1's DVE/PE work must be
            # emitted BEFORE image 0's folds or it stalls behind them) ----
            phase_pre(0)
            phase_sel(0, range(8))
            phase_qp(0)
            phase_wrap(0)
            phase_pre(1)
            phase_gather_dma(0, 0)
            phase_fold(0, 0)
            phase_sel(1, range(0, 4))
            phase_gather_dma(0, 1)
            phase_fold(0, 1)
            phase_sel(1, range(4, 8))
            phase_gather_dma(0, 2)
            phase_fold(0, 2)
            phase_qp(1)
            phase_wrap(1)
            phase_gather_dma(0, 3)
            phase_fold(0, 3)
            phase_edge(0)
            phase_g2(0)
            phase_gather_dma(1, 0)
            phase_fold(1, 0)
            phase_f1(0)
            phase_gather_dma(1, 1)
            phase_fold(1, 1)
            phase_f2(0)
            phase_gather_dma(1, 2)
            phase_fold(1, 2)
            phase_bneck(0)
            phase_gather_dma(1, 3)
            phase_fold(1, 3)
            phase_out(0)
            phase_edge(1)
            phase_g2(1)
            phase_f1(1, dve_evac=True)
            phase_f2(1)
            phase_bneck(1)
            phase_out(1)

    nc.finalize()
    return nc


# --------------------------------------------------------------------------
# entry point
# --------------------------------------------------------------------------
def kernel(**inputs):
    inp = {k: np.asarray(v) for k, v in inputs.items()}
    w = _prep_weights(inp)

    if 'nc' not in _cache:
        _cache['nc'] = _build_bass()
    nc = _cache['nc']

    xh = inp['x'].astype(np.float16).reshape(B, C, N)
    in_maps = []
    for c in range(N_CORES):
        m = {'xh': np.ascontiguousarray(xh[c * 2:(c + 1) * 2])}
        m.update(w)
        in_maps.append(m)

    from concourse.bass_utils import run_bass_kernel_spmd
    trace = bool(os.environ.get("KBENCH_TRACE"))
    res = run_bass_kernel_spmd(nc, in_maps, core_ids=list(range(N_CORES)),
                               trace=trace)
    _cache['exec_time_ns'] = res.exec_time_ns
    _cache['results'] = res
    out = np.zeros((B, C, N), np.float32)
    for c in range(N_CORES):
        out[c * 2:(c + 1) * 2] = res.results[c]['out']
    return out.reshape(B, C, H, W)


# revision 14
# speedup vs baseline: 1.8683x; 1.0146x over previous
"""Trainium2 Bass kernel for nn_Block_17033840296551 (GNN message passing block).

Data-parallel over batch: 16 images -> 8 cores x 2 images. Each core runs the
full block (g1 conv -> kNN top-8+self -> EdgeConv max-agg -> g2 -> FFN ->
bottleneck -> final BN) on its 2 images with no cross-core communication.

v3 (gather + overlap rewrite):
  * NON-transposed dma_gather (n-major output): the transposed gather's rx
    side emits one 256B xbar-spray descriptor per 256 payload bytes (4x the
    descriptors), which made each 2k-idx gather ~20us of gpsimd descgen.
    n-major gathers are ~2x cheaper on the Q7s. The EdgeConv max-fold runs
    in n-major layout (where the self term is q_sb itself, no qT needed);
    e = relu(p+maxq) is then transposed to ch-major via 32 PE tile
    transposes for the g2 matmul.
  * Two-image software pipeline by construction: per-image tile pools AND
    per-image PSUM pools (a shared psum pool's slot rotation chained image
    1's first matmul behind image 0's last evacuation), with emission
    interleaved so image 1's compute phases sit inside image 0's gather
    window and vice versa.
  * All residual adds (g2, f2, b3, outer) ride the PSUM accumulation as
    identity matmuls; final BN fused into the last evacuation (scale/bias).
  * EdgeConv bias bp enters as a K=1 matmul (ones row x bp row) since it is
    per-channel and channels sit on the free axis in n-major layout.
  * Norms: n2 broadcast to 128 partitions via all-ones matmul, rsqrt via
    reciprocal_approx_fast + Sqrt activation.
"""

import os
import numpy as np

# problem constants (hardcoded per harness contract)
B, C, H, W = 16, 256, 32, 32
N = H * W           # 1024 pixels per image
K = 9
EPS = 1e-5
IMGS_PER_CORE = 2
N_CORES = 8
NEG_BIG = -30000.0

_cache = {}


# --------------------------------------------------------------------------
# host-side preprocessing
# --------------------------------------------------------------------------
def _bn_fold(p):
    g, b, m, v = np.asarray(p, np.float32)
    s = g / np.sqrt(v + EPS)
    t = b - m * s
    return s, t


def _pack_kxm(w_t, part=128):
    """[K, M] -> [part, K//part, M] (partition-major K tiling)."""
    Kd, M = w_t.shape
    kt = Kd // part
    return np.ascontiguousarray(w_t.reshape(kt, part, M).transpose(1, 0, 2))


def _pack_bias(b, part=128):
    n = b.shape[0]
    t = n // part
    return np.ascontiguousarray(b.reshape(t, part).T)  # [part, t]


def _prep_weights(inp):
    f16 = np.float16
    s1, t1 = _bn_fold(inp['g1_bn'])
    Wg1 = s1[:, None] * inp['g1_w']
    s2, t2 = _bn_fold(inp['g2_bn'])
    Wg2 = s2[:, None] * inp['g2_w']
    sf1, tf1 = _bn_fold(inp['f1_bn'])
    Wf1 = sf1[:, None] * inp['f1_w']
    bf1 = sf1 * inp['f1_b'] + tf1
    sf2, tf2 = _bn_fold(inp['f2_bn'])
    Wf2 = sf2[:, None] * inp['f2_w']
    bf2 = sf2 * inp['f2_b'] + tf2
    sb1, tb1 = _bn_fold(inp['b1_bn'])
    Wb1 = sb1[:, None] * inp['b1_w']
    sb2, tb2 = _bn_fold(inp['b2_bn'])
    Wb2 = sb2[:, None, None, None] * inp['b2_w']
    sb3, tb3 = _bn_fold(inp['b3_bn'])
    Wb3 = sb3[:, None] * inp['b3_w']
    sf, tf = _bn_fold(inp['bnf'])
    # final BN absorbs b3's bias: out = sf*(P + tb3) + tf, P = b3conv+h2+x
    btf2 = sf * tb3 + tf

    A = inp['edge_w'][:, :C]
    Bm = inp['edge_w'][:, C:]
    Wp = A - Bm
    Wq = Bm
    bp = inp['edge_b']

    wb2_t = np.zeros((64, 9, 64), f16)
    for dy in range(3):
        for dx in range(3):
            wb2_t[:, dy * 3 + dx, :] = Wb2[:, :, dy, dx].T.astype(f16)

    return {
        'wg1': _pack_kxm(Wg1.T.astype(f16)),           # [128,2,256]
        'wp': _pack_kxm(Wp.T.astype(f16)),             # [128,2,512]
        'wq': _pack_kxm(Wq.T.astype(f16)),             # [128,2,512]
        'wg2': _pack_kxm(Wg2.T.astype(f16)),           # [128,4,256]
        'wf1': _pack_kxm(Wf1.T.astype(f16)),           # [128,2,1024]
        'wf2': _pack_kxm(Wf2.T.astype(f16)),           # [128,8,256]
        'wb1': _pack_kxm(Wb1.T.astype(f16)),           # [128,2,64]
        'wb2': wb2_t,                                   # [64,9,64]
        'wb3': Wb3.T.astype(f16),                       # [64,256]
        'bt1': _pack_bias(t1),                          # [128,2] f32
        'bt2': _pack_bias(t2),
        'bpv': np.ascontiguousarray(bp.astype(f16).reshape(1, 512)),
        'bbf1': _pack_bias(bf1),                        # [128,8]
        'bbf2': _pack_bias(bf2),
        'btb1': np.ascontiguousarray(tb1[:, None].astype(np.float32)),  # [64,1]
        'btb2': np.ascontiguousarray(tb2[:, None].astype(np.float32)),
        'bsf': _pack_bias(sf),
        'btf2': _pack_bias(btf2),
    }


# --------------------------------------------------------------------------
# device kernel builder
# --------------------------------------------------------------------------
def _build_bass():
    import concourse.bass as bass
    import concourse.mybir as mybir
    from concourse import bacc
    from concourse.tile import TileContext
    from concourse.masks import make_identity

    dt = mybir.dt
    F16 = dt.float16
    F32 = dt.float32
    AF = mybir.ActivationFunctionType

    nc = bacc.Bacc()

    # ---- DRAM parameters ----
    xh_d = nc.declare_dram_parameter("xh", [IMGS_PER_CORE, C, N], F16, isOutput=False)
    wg1_d = nc.declare_dram_parameter("wg1", [128, 2, 256], F16, isOutput=False)
    wp_d = nc.declare_dram_parameter("wp", [128, 2, 512], F16, isOutput=False)
    wq_d = nc.declare_dram_parameter("wq", [128, 2, 512], F16, isOutput=False)
    wg2_d = nc.declare_dram_parameter("wg2", [128, 4, 256], F16, isOutput=False)
    wf1_d = nc.declare_dram_parameter("wf1", [128, 2, 1024], F16, isOutput=False)
    wf2_d = nc.declare_dram_parameter("wf2", [128, 8, 256], F16, isOutput=False)
    wb1_d = nc.declare_dram_parameter("wb1", [128, 2, 64], F16, isOutput=False)
    wb2_d = nc.declare_dram_parameter("wb2", [64, 9, 64], F16, isOutput=False)
    wb3_d = nc.declare_dram_parameter("wb3", [64, 256], F16, isOutput=False)
    bt1_d = nc.declare_dram_parameter("bt1", [128, 2], F32, isOutput=False)
    bt2_d = nc.declare_dram_parameter("bt2", [128, 2], F32, isOutput=False)
    bpv_d = nc.declare_dram_parameter("bpv", [1, 512], F16, isOutput=False)
    bbf1_d = nc.declare_dram_parameter("bbf1", [128, 8], F32, isOutput=False)
    bbf2_d = nc.declare_dram_parameter("bbf2", [128, 2], F32, isOutput=False)
    btb1_d = nc.declare_dram_parameter("btb1", [64, 1], F32, isOutput=False)
    btb2_d = nc.declare_dram_parameter("btb2", [64, 1], F32, isOutput=False)
    bsf_d = nc.declare_dram_parameter("bsf", [128, 2], F32, isOutput=False)
    btf2_d = nc.declare_dram_parameter("btf2", [128, 2], F32, isOutput=False)
    q_drams = [nc.dram_tensor(f"q_dram{i}", [N, 512], F16)
               for i in range(IMGS_PER_CORE)]
    out_d = nc.declare_dram_parameter("out", [IMGS_PER_CORE, C, N], F32, isOutput=True)

    with TileContext(nc) as tc:
        import contextlib
        ctx = contextlib.ExitStack()
        with ctx:
            consts = ctx.enter_context(tc.tile_pool(name="consts", bufs=1))
            # per-image pools: every tag is a dedicated buffer; PSUM pools are
            # per-image so slot rotation never chains one image behind the other
            pools = [ctx.enter_context(tc.tile_pool(name=f"im{i}", bufs=1))
                     for i in range(IMGS_PER_CORE)]
            psums = [ctx.enter_context(
                tc.tile_pool(name=f"psum{i}", bufs=3, space="PSUM"))
                for i in range(IMGS_PER_CORE)]
            psum64s = [ctx.enter_context(
                tc.tile_pool(name=f"psum64_{i}", bufs=1, space="PSUM"))
                for i in range(IMGS_PER_CORE)]
            pool_gath = ctx.enter_context(tc.tile_pool(name="gath", bufs=2))

            # ---- constants / weights (loaded once) ----
            # alternate sync/scalar HWDGE queues so ~20 loads don't serialize
            # in front of the first matmul
            _ldq = [0]

            def load(name, shape, dtype, src, eng=None):
                t = consts.tile(shape, dtype, name=name)
                if eng is None:
                    eng = nc.sync if _ldq[0] % 2 == 0 else nc.scalar
                    _ldq[0] += 1
                eng.dma_start(out=t[:], in_=src[:])
                return t

            wg1 = load("wg1s", [128, 2, 256], F16, wg1_d, eng=nc.sync)
            bt1 = load("bt1s", [128, 2], F32, bt1_d, eng=nc.scalar)
            ones = consts.tile([128, 128], F16, name="ones")
            nc.gpsimd.memset(ones[:], 1.0)
            wp = load("wps", [128, 2, 512], F16, wp_d)
            wq = load("wqs", [128, 2, 512], F16, wq_d)
            wg2 = load("wg2s", [128, 4, 256], F16, wg2_d)
            wf1 = load("wf1s", [128, 2, 1024], F16, wf1_d)
            wf2 = load("wf2s", [128, 8, 256], F16, wf2_d)
            wb1 = load("wb1s", [128, 2, 64], F16, wb1_d)
            wb2 = load("wb2s", [64, 9, 64], F16, wb2_d)
            wb3 = load("wb3s", [64, 256], F16, wb3_d)
            bt2 = load("bt2s", [128, 2], F32, bt2_d)
            bpv = load("bpvs", [1, 512], F16, bpv_d)
            bbf1 = load("bbf1s", [128, 8], F32, bbf1_d)
            bbf2 = load("bbf2s", [128, 2], F32, bbf2_d)
            btb1 = load("btb1s", [64, 1], F32, btb1_d)
            btb2 = load("btb2s", [64, 1], F32, btb2_d)
            bsf = load("bsfs", [128, 2], F32, bsf_d)
            btf2 = load("btf2s", [128, 2], F32, btf2_d)

            ident = consts.tile([128, 128], F16, name="ident")
            make_identity(nc, ident[:])
            negid = consts.tile([128, 128], F16, name="negid")
            nc.scalar.activation(out=negid[:], in_=ident[:], func=AF.Copy,
                                 scale=NEG_BIG)
            # idbig[k, f] = 1 iff f == k + 384 (shifted identity for diag-kill)
            idbig = consts.tile([128, 1024], F16, name="idbig")
            nc.gpsimd.memset(idbig[:], 0.0)
            nc.gpsimd.affine_select(
                out=idbig[:], in_=idbig[:],
                compare_op=mybir.AluOpType.not_equal, fill=1.0,
                base=384, pattern=[[-1, 1024]], channel_multiplier=1)
            epsb = consts.tile([128, 1], F32, name="epsb")
            nc.gpsimd.memset(epsb[:], 1e-12)
            # dummy gather: loads the gather ucode into Q7 IRAM now instead of
            # in front of image 0's first real gather
            zidx = consts.tile([128, 8], dt.int16, name="zidx")
            nc.gpsimd.memset(zidx[:], 0)
            scrg = consts.tile([128, 1, 512], F16, name="scrg")
            nc.gpsimd.dma_gather(
                out_ap=scrg[:], in_ap=q_drams[0][:], idxs_ap=zidx[:],
                num_idxs=128, num_idxs_reg=128, elem_size=512,
                transpose=False, single_packet=False)

            # per-image tile state
            st = [{} for _ in range(IMGS_PER_CORE)]

            def T(img, tag, shape, dtype):
                t = pools[img].tile(shape, dtype, name=f"{tag}_{img}", tag=tag)
                st[img][tag] = t
                return t

            def phase_pre(img):
                """load x, g1 conv, feature norms."""
                xh = T(img, "xh", [128, 2, N], F16)
                for t in range(2):
                    nc.sync.dma_start(out=xh[:, t, :],
                                      in_=xh_d[img, t * 128:(t + 1) * 128, :])
                featT = T(img, "feat", [128, 2, N], F16)
                for to in range(2):
                    pss = [psums[img].tile([128, 512], F32,
                                           name=f"ps_g1_{img}_{to}_{nb}", tag="ps")
                           for nb in range(2)]
                    for kt in range(2):
                        for nb in range(2):
                            nc.tensor.matmul(
                                pss[nb][:], lhsT=wg1[:, kt, to * 128:(to + 1) * 128],
                                rhs=xh[:, kt, nb * 512:(nb + 1) * 512],
                                start=(kt == 0), stop=(kt == 1))
                    for nb in range(2):
                        nc.scalar.activation(
                            out=featT[:, to, nb * 512:(nb + 1) * 512], in_=pss[nb][:],
                            func=AF.Identity, bias=bt1[:, to:to + 1])
                fsq = T(img, "fx1", [128, 2, N], F16)
                for t in range(2):
                    nc.vector.tensor_mul(fsq[:, t, :], featT[:, t, :], featT[:, t, :])
                n2b = T(img, "n2b", [128, N], F32)
                for nb in range(2):
                    psn = psums[img].tile([128, 512], F32,
                                          name=f"ps_n2_{img}_{nb}", tag="ps")
                    for kt in range(2):
                        nc.tensor.matmul(
                            psn[:], lhsT=ones[:],
                            rhs=fsq[:, kt, nb * 512:(nb + 1) * 512],
                            start=(kt == 0), stop=(kt == 1))
                    nc.scalar.activation(out=n2b[:, nb * 512:(nb + 1) * 512],
                                         in_=psn[:], func=AF.Identity,
                                         bias=epsb[:, 0:1])
                rn2 = T(img, "fx1", [128, N], F32)  # reuses fsq slot (fsq dead)
                nc.vector.reciprocal_approx_fast(out=rn2[:], in_=n2b[:])
                invnb = T(img, "invnb", [128, N], F16)
                nc.scalar.activation(out=invnb[:], in_=rn2[:], func=AF.Sqrt)
                xnT = T(img, "n2b", [128, 2, N], F16)  # reuses n2b slot
                for t in range(2):
                    nc.vector.tensor_mul(xnT[:, t, :], featT[:, t, :], invnb[:])

            def phase_sel(img, Is):
                """sim I-blocks + top-8 selection for I in Is."""
                featT = st[img]["feat"]
                xnT = st[img]["n2b"]
                if 0 in Is:
                    st[img]["ixt"] = T(img, "ix", [128, 4, 8, 2], dt.uint16)
                ixbuf = st[img]["ixt"]
                for I in Is:
                    simblk = pools[img].tile([128, N], F16, name=f"sim{img}_{I}",
                                             tag="sim", bufs=2)
                    pss = [psums[img].tile([128, 512], F32,
                                           name=f"ps_sim_{img}_{I}_{cb}", tag="ps")
                           for cb in range(2)]
                    for kt in range(2):
                        for cb in range(2):
                            has_diag = (cb == I // 4)
                            nc.tensor.matmul(
                                pss[cb][:], lhsT=featT[:, kt, I * 128:(I + 1) * 128],
                                rhs=xnT[:, kt, cb * 512:(cb + 1) * 512],
                                start=(kt == 0),
                                stop=(kt == 1 and not has_diag))
                    for cb in range(2):
                        if cb == I // 4:
                            off = I * 128 - cb * 512
                            nc.tensor.matmul(pss[cb][:], lhsT=negid[:],
                                             rhs=idbig[:, 384 - off:896 - off],
                                             start=False, stop=True)
                        nc.scalar.activation(
                            out=simblk[:, cb * 512:(cb + 1) * 512], in_=pss[cb][:],
                            func=AF.Copy)
                    mx = pools[img].tile([128, 8], F16, name=f"mx{img}_{I}",
                                         tag="mx", bufs=2)
                    nc.vector.max(out=mx[:], in_=simblk[:])
                    nc.vector.max_index(out=ixbuf[:, I // 2, :, I % 2],
                                        in_max=mx[:], in_values=simblk[:])

            def phase_wrap(img):
                """wrapped idx buffer [128, 512] i16.
                col = 128*s + 16*k + 8*i + g ; idx number within s-block
                = 256*k + 128*i + 16*g + p16 -> node 256*s + 128*i + 16*g + p16
                """
                wrapped = T(img, "wrap", [128, 512], dt.int16)
                wtmp = T(img, "wtmp", [16, 8, 64], dt.int16)
                ixi = st[img]["ix"][:].bitcast(dt.int16)
                ixf = ixi.rearrange("p s k i -> p (s k i)")
                for g in range(8):
                    nc.sync.dma_start(out=wtmp[:, g, :],
                                      in_=ixf[16 * g:16 * (g + 1), :])
                # per-partition (g,s,k,i) -> (s,k,i,g) permute on DVE
                nc.vector.tensor_copy(
                    wrapped[0:16, :].rearrange(
                        "p (s k i g) -> p s k i g", s=4, k=8, i=2, g=8),
                    wtmp[:].rearrange("p g (s k i) -> p s k i g", s=4, k=8, i=2))
                nc.sync.dma_start(out=wrapped[16:32, :], in_=wrapped[0:16, :])
                nc.sync.dma_start(out=wrapped[32:64, :], in_=wrapped[0:32, :])
                nc.sync.dma_start(out=wrapped[64:128, :], in_=wrapped[0:64, :])

            def phase_qp(img):
                """q (n-major, to DRAM for gather) and p (n-major, +bp bias)."""
                featT = st[img]["feat"]
                q_sb = T(img, "q", [128, 8, 512], F16)
                q_dram = q_drams[img]
                for nt in range(8):
                    ps = psums[img].tile([128, 512], F32,
                                         name=f"ps_q_{img}_{nt}", tag="ps")
                    for kt in range(2):
                        nc.tensor.matmul(
                            ps[:], lhsT=featT[:, kt, nt * 128:(nt + 1) * 128],
                            rhs=wq[:, kt, :], start=(kt == 0), stop=(kt == 1))
                    nc.scalar.activation(out=q_sb[:, nt, :], in_=ps[:], func=AF.Copy)
                    nc.sync.dma_start(out=q_dram[nt * 128:(nt + 1) * 128, :],
                                      in_=q_sb[:, nt, :])
                p_nm = T(img, "pT", [128, 8, 512], F16)
                for nt in range(8):
                    ps = psums[img].tile([128, 512], F32,
                                         name=f"ps_p_{img}_{nt}", tag="ps")
                    for kt in range(2):
                        nc.tensor.matmul(
                            ps[:], lhsT=featT[:, kt, nt * 128:(nt + 1) * 128],
                            rhs=wp[:, kt, :], start=(kt == 0), stop=False)
                    # per-channel EdgeConv bias via K=1 broadcast matmul
                    nc.tensor.matmul(ps[:], lhsT=ones[0:1, :], rhs=bpv[0:1, :],
                                     start=False, stop=True)
                    nc.scalar.activation(out=p_nm[:, nt, :], in_=ps[:], func=AF.Copy)

            def phase_gather_dma(img, s):
                """issue the neighbor gather for s-block (n-major output)."""
                if s == 0:
                    st[img]["maxqt"] = T(img, "maxq", [128, 8, 512], F16)
                wrapped = st[img]["wrap"]
                go = pool_gath.tile([128, 16, 512], F16, name=f"go{img}_{s}",
                                    tag="go")
                st[img][f"go{s}"] = go
                nc.gpsimd.dma_gather(
                    out_ap=go[:], in_ap=q_drams[img][:],
                    idxs_ap=wrapped[:, 128 * s:128 * (s + 1)],
                    num_idxs=2048, num_idxs_reg=2048, elem_size=512,
                    transpose=False, single_packet=False)

            def phase_fold(img, s):
                """8-way max fold + self term for s-block."""
                maxq = st[img]["maxqt"]
                q_sb = st[img]["q"]
                go = st[img][f"go{s}"]
                gv = go[:].rearrange("p (k h) c -> p k h c", k=8)
                nc.vector.tensor_max(gv[:, 4:8, :, :], gv[:, 0:4, :, :],
                                     gv[:, 4:8, :, :])
                nc.vector.tensor_max(gv[:, 6:8, :, :], gv[:, 4:6, :, :],
                                     gv[:, 6:8, :, :])
                nc.vector.tensor_max(gv[:, 7, :, :], gv[:, 6, :, :],
                                     gv[:, 7, :, :])
                nc.vector.tensor_max(maxq[:, 2 * s:2 * s + 2, :],
                                     gv[:, 7, :, :], q_sb[:, 2 * s:2 * s + 2, :])

            def phase_edge(img):
                """e = relu(p + maxq) in n-major, then PE-transpose to
                ch-major eT [128, 4, N]."""
                p_nm = st[img]["pT"]
                maxq = st[img]["maxqt"]
                e_nm = T(img, "n2b", [128, 8, 512], F16)  # reuses xnT slot
                flat = [t[:].rearrange("p a c -> p (a c)")
                        for t in (e_nm, p_nm, maxq)]
                nc.vector.tensor_add(flat[0], flat[1], flat[2])
                # relu is applied by the transpose evacuation below
                eT = T(img, "q", [128, 4, N], F16)  # reuses q slot (q dead)
                for a in range(4):
                    for nb in range(2):
                        ps = psums[img].tile([128, 512], F16,
                                             name=f"ps_tr_{img}_{a}_{nb}", tag="ps")
                        for j in range(4):
                            nc.tensor.transpose(
                                out=ps[:, j * 128:(j + 1) * 128],
                                in_=e_nm[:, 4 * nb + j, a * 128:(a + 1) * 128],
                                identity=ident[:])
                        nc.scalar.activation(
                            out=eT[:, a, nb * 512:(nb + 1) * 512], in_=ps[:],
                            func=AF.Relu)

            def phase_g2(img, dve_evac=False):
                """g2 conv + residual (ident@xh in PSUM) -> hc f16."""
                eT = st[img]["q"]
                xh = st[img]["xh"]
                hc = T(img, "hc", [128, 2, N], F16)
                for to in range(2):
                    pss = [psums[img].tile([128, 512], F32,
                                           name=f"ps_g2_{img}_{to}_{nb}", tag="ps")
                           for nb in range(2)]
                    for kt in range(4):
                        for nb in range(2):
                            nc.tensor.matmul(
                                pss[nb][:], lhsT=wg2[:, kt, to * 128:(to + 1) * 128],
                                rhs=eT[:, kt, nb * 512:(nb + 1) * 512],
                                start=(kt == 0), stop=False)
                    for nb in range(2):
                        nc.tensor.matmul(
                            pss[nb][:], lhsT=ident[:],
                            rhs=xh[:, to, nb * 512:(nb + 1) * 512],
                            start=False, stop=True)
                        if dve_evac and nb == 1:
                            nc.vector.tensor_scalar_add(
                                hc[:, to, nb * 512:(nb + 1) * 512], pss[nb][:],
                                bt2[:, to:to + 1])
                        else:
                            nc.scalar.activation(
                                out=hc[:, to, nb * 512:(nb + 1) * 512],
                                in_=pss[nb][:],
                                func=AF.Identity, bias=bt2[:, to:to + 1])

            def phase_f1(img, dve_evac=False):
                hc = st[img]["hc"]
                f1o = T(img, "f1o", [128, 8, N], F16)
                for to in range(8):
                    pss = [psums[img].tile([128, 512], F32,
                                           name=f"ps_f1_{img}_{to}_{nb}", tag="ps")
                           for nb in range(2)]
                    for kt in range(2):
                        for nb in range(2):
                            nc.tensor.matmul(
                                pss[nb][:], lhsT=wf1[:, kt, to * 128:(to + 1) * 128],
                                rhs=hc[:, kt, nb * 512:(nb + 1) * 512],
                                start=(kt == 0), stop=(kt == 1))
                    for nb in range(2):
                        dst = f1o[:, to, nb * 512:(nb + 1) * 512]
                        if dve_evac and to % 2 == 1:
                            nc.vector.tensor_scalar(
                                out=dst, in0=pss[nb][:],
                                scalar1=bbf1[:, to:to + 1], scalar2=0.0,
                                op0=mybir.AluOpType.add,
                                op1=mybir.AluOpType.max)
                        else:
                            nc.scalar.activation(
                                out=dst, in_=pss[nb][:],
                                func=AF.Relu, bias=bbf1[:, to:to + 1])

            def phase_f2(img, dve_evac=False):
                f1o = st[img]["f1o"]
                hc = st[img]["hc"]
                h2c = T(img, "pT", [128, 2, N], F16)  # reuses p slot (p dead)
                for to in range(2):
                    pss = [psums[img].tile([128, 512], F32,
                                           name=f"ps_f2_{img}_{to}_{nb}", tag="ps")
                           for nb in range(2)]
                    for kt in range(8):
                        for nb in range(2):
                            nc.tensor.matmul(
                                pss[nb][:], lhsT=wf2[:, kt, to * 128:(to + 1) * 128],
                                rhs=f1o[:, kt, nb * 512:(nb + 1) * 512],
                                start=(kt == 0), stop=False)
                    for nb in range(2):
                        nc.tensor.matmul(
                            pss[nb][:], lhsT=ident[:],
                            rhs=hc[:, to, nb * 512:(nb + 1) * 512],
                            start=False, stop=True)
                        if dve_evac and nb == 1:
                            nc.vector.tensor_scalar_add(
                                h2c[:, to, nb * 512:(nb + 1) * 512], pss[nb][:],
                                bbf2[:, to:to + 1])
                        else:
                            nc.scalar.activation(
                                out=h2c[:, to, nb * 512:(nb + 1) * 512],
                                in_=pss[nb][:],
                                func=AF.Identity, bias=bbf2[:, to:to + 1])

            def phase_bneck(img):
                h2c = st[img]["pT"]
                pad = T(img, "pad", [64, 34 * 34], F16)
                nc.vector.memset(pad[:], 0.0)
                pad3 = pad[:].rearrange("p (r c) -> p r c", r=34)
                for nb in range(2):
                    ps = psum64s[img].tile([64, 512], F32,
                                           name=f"ps_b1_{img}_{nb}", tag="ps64")
                    for kt in range(2):
                        nc.tensor.matmul(
                            ps[:], lhsT=wb1[:, kt, :],
                            rhs=h2c[:, kt, nb * 512:(nb + 1) * 512],
                            start=(kt == 0), stop=(kt == 1))
                    # evacuate straight into the zero-padded conv input
                    nc.scalar.activation(
                        out=pad3[:, 1 + 16 * nb:17 + 16 * nb, 1:33],
                        in_=ps[:].rearrange("p (r c) -> p r c", r=16),
                        func=AF.Relu, bias=btb1[:, 0:1])
                b2o = T(img, "b2o", [64, N], F16)
                for nb in range(2):
                    ps = psum64s[img].tile([64, 512], F32,
                                           name=f"ps_b2_{img}_{nb}", tag="ps64")
                    for tap in range(9):
                        dy, dx = tap // 3, tap % 3
                        rhs = pad3[:, 16 * nb + dy:16 * nb + dy + 16, dx:dx + 32]
                        nc.tensor.matmul(ps[:], lhsT=wb2[:, tap, :], rhs=rhs,
                                         start=(tap == 0), stop=(tap == 8))
                    nc.scalar.activation(out=b2o[:, nb * 512:(nb + 1) * 512],
                                         in_=ps[:], func=AF.Relu, bias=btb2[:, 0:1])

            def phase_out(img, dve_evac=False):
                """b3 + h2 + x residuals in PSUM; final BN in the evacuation."""
                h2c = st[img]["pT"]
                xh = st[img]["xh"]
                b2o = st[img]["b2o"]
                out32 = T(img, "maxq", [128, 2, N], F32)  # reuses maxq slot
                for to in range(2):
                    pss = [psums[img].tile([128, 512], F32,
                                           name=f"ps_b3_{img}_{to}_{nb}", tag="ps")
                           for nb in range(2)]
                    for nb in range(2):
                        sl = slice(nb * 512, (nb + 1) * 512)
                        nc.tensor.matmul(
                            pss[nb][:], lhsT=wb3[:, to * 128:(to + 1) * 128],
                            rhs=b2o[:, sl], start=True, stop=False)
                        nc.tensor.matmul(
                            pss[nb][:], lhsT=ident[:], rhs=h2c[:, to, sl],
                            start=False, stop=False)
                        nc.tensor.matmul(
                            pss[nb][:], lhsT=ident[:], rhs=xh[:, to, sl],
                            start=False, stop=True)
                        if dve_evac and nb == 1:
                            nc.vector.tensor_scalar(
                                out=out32[:, to, sl], in0=pss[nb][:],
                                scalar1=bsf[:, to:to + 1],
                                scalar2=btf2[:, to:to + 1],
                                op0=mybir.AluOpType.mult,
                                op1=mybir.AluOpType.add)
                        else:
                            nc.scalar.activation(
                                out=out32[:, to, sl], in_=pss[nb][:],
                                func=AF.Identity, scale=bsf[:, to:to + 1],
                                bias=btf2[:, to:to + 1])
                    for nb in range(2):
                        nc.sync.dma_start(
                            out=out_d[img, to * 128:(to + 1) * 128,
                                      nb * 512:(nb + 1) * 512],
                            in_=out32[:, to, nb * 512:(nb + 1) * 512])

            # ---- two-image software pipeline (emission order = priority;
            # engine queues execute in order, so image 1's DVE/PE work must be
            # emitted BEFORE image 0's folds or it stalls behind them) ----
            phase_pre(0)
            phase_sel(0, range(8))
            phase_qp(0)
            phase_wrap(0)
            phase_gather_dma(0, 0)
            phase_pre(1)
            phase_fold(0, 0)
            phase_sel(1, range(0, 4))
            phase_gather_dma(0, 1)
            phase_fold(0, 1)
            phase_sel(1, range(4, 8))
            phase_gather_dma(0, 2)
            phase_fold(0, 2)
            phase_qp(1)
            phase_wrap(1)
            phase_gather_dma(0, 3)
            phase_fold(0, 3)
            phase_edge(0)
            phase_g2(0)
            phase_gather_dma(1, 0)
            phase_fold(1, 0)
            phase_f1(0)
            phase_gather_dma(1, 1)
            phase_fold(1, 1)
            phase_f2(0)
            phase_gather_dma(1, 2)
            phase_fold(1, 2)
            phase_bneck(0)
            phase_gather_dma(1, 3)
            phase_fold(1, 3)
            phase_out(0)
            phase_edge(1)
            phase_g2(1, dve_evac=True)
            phase_f1(1, dve_evac=True)
            phase_f2(1, dve_evac=True)
            phase_bneck(1)
            phase_out(1, dve_evac=True)

    nc.finalize()
    return nc


# --------------------------------------------------------------------------
# entry point
# --------------------------------------------------------------------------
def kernel(**inputs):
    inp = {k: np.asarray(v) for k, v in inputs.items()}
    w = _prep_weights(inp)

    if 'nc' not in _cache:
        _cache['nc'] = _build_bass()
    nc = _cache['nc']

    xh = inp['x'].astype(np.float16).reshape(B, C, N)
    in_maps = []
    for c in range(N_CORES):
        m = {'xh': np.ascontiguousarray(xh[c * 2:(c + 1) * 2])}
        m.update(w)
        in_maps.append(m)

    from concourse.bass_utils import run_bass_kernel_spmd
    trace = bool(os.environ.get("KBENCH_TRACE"))
    res = run_bass_kernel_spmd(nc, in_maps, core_ids=list(range(N_CORES)),
                               trace=trace)
    _cache['exec_time_ns'] = res.exec_time_ns
    _cache['results'] = res
    out = np.zeros((B, C, N), np.float32)
    for c in range(N_CORES):
        out[c * 2:(c + 1) * 2] = res.results[c]['out']
    return out.reshape(B, C, H, W)


# revision 19
# speedup vs baseline: 1.9326x; 1.0344x over previous
"""Trainium2 Bass kernel for nn_Block_17033840296551 (GNN message passing block).

Data-parallel over batch: 16 images -> 8 cores x 2 images. Each core runs the
full block (g1 conv -> kNN top-8+self -> EdgeConv max-agg -> g2 -> FFN ->
bottleneck -> final BN) on its 2 images with no cross-core communication.

v3 (gather + overlap rewrite):
  * NON-transposed dma_gather (n-major output): the transposed gather's rx
    side emits one 256B xbar-spray descriptor per 256 payload bytes (4x the
    descriptors), which made each 2k-idx gather ~20us of gpsimd descgen.
    n-major gathers are ~2x cheaper on the Q7s. The EdgeConv max-fold runs
    in n-major layout (where the self term is q_sb itself, no qT needed);
    e = relu(p+maxq) is then transposed to ch-major via 32 PE tile
    transposes for the g2 matmul.
  * Two-image software pipeline by construction: per-image tile pools AND
    per-image PSUM pools (a shared psum pool's slot rotation chained image
    1's first matmul behind image 0's last evacuation), with emission
    interleaved so image 1's compute phases sit inside image 0's gather
    window and vice versa.
  * All residual adds (g2, f2, b3, outer) ride the PSUM accumulation as
    identity matmuls; final BN fused into the last evacuation (scale/bias).
  * EdgeConv bias bp enters as a K=1 matmul (ones row x bp row) since it is
    per-channel and channels sit on the free axis in n-major layout.
  * Norms: n2 broadcast to 128 partitions via all-ones matmul, rsqrt via
    reciprocal_approx_fast + Sqrt activation.
"""

import os
import numpy as np

# problem constants (hardcoded per harness contract)
B, C, H, W = 16, 256, 32, 32
N = H * W           # 1024 pixels per image
K = 9
EPS = 1e-5
IMGS_PER_CORE = 2
N_CORES = 8
NEG_BIG = -30000.0

_cache = {}


# --------------------------------------------------------------------------
# host-side preprocessing
# --------------------------------------------------------------------------
def _bn_fold(p):
    g, b, m, v = np.asarray(p, np.float32)
    s = g / np.sqrt(v + EPS)
    t = b - m * s
    return s, t


def _pack_kxm(w_t, part=128):
    """[K, M] -> [part, K//part, M] (partition-major K tiling)."""
    Kd, M = w_t.shape
    kt = Kd // part
    return np.ascontiguousarray(w_t.reshape(kt, part, M).transpose(1, 0, 2))


def _pack_bias(b, part=128):
    n = b.shape[0]
    t = n // part
    return np.ascontiguousarray(b.reshape(t, part).T)  # [part, t]


def _prep_weights(inp):
    f16 = np.float16
    s1, t1 = _bn_fold(inp['g1_bn'])
    Wg1 = s1[:, None] * inp['g1_w']
    s2, t2 = _bn_fold(inp['g2_bn'])
    Wg2 = s2[:, None] * inp['g2_w']
    sf1, tf1 = _bn_fold(inp['f1_bn'])
    Wf1 = sf1[:, None] * inp['f1_w']
    bf1 = sf1 * inp['f1_b'] + tf1
    sf2, tf2 = _bn_fold(inp['f2_bn'])
    Wf2 = sf2[:, None] * inp['f2_w']
    bf2 = sf2 * inp['f2_b'] + tf2
    sb1, tb1 = _bn_fold(inp['b1_bn'])
    Wb1 = sb1[:, None] * inp['b1_w']
    sb2, tb2 = _bn_fold(inp['b2_bn'])
    Wb2 = sb2[:, None, None, None] * inp['b2_w']
    sb3, tb3 = _bn_fold(inp['b3_bn'])
    Wb3 = sb3[:, None] * inp['b3_w']
    sf, tf = _bn_fold(inp['bnf'])
    # final BN absorbs b3's bias: out = sf*(P + tb3) + tf, P = b3conv+h2+x
    btf2 = sf * tb3 + tf

    A = inp['edge_w'][:, :C]
    Bm = inp['edge_w'][:, C:]
    Wp = A - Bm
    Wq = Bm
    bp = inp['edge_b']

    wb2_t = np.zeros((64, 9, 64), f16)
    for dy in range(3):
        for dx in range(3):
            wb2_t[:, dy * 3 + dx, :] = Wb2[:, :, dy, dx].T.astype(f16)

    return {
        'wg1': _pack_kxm(Wg1.T.astype(f16)),           # [128,2,256]
        'wp': _pack_kxm(Wp.T.astype(f16)),             # [128,2,512]
        'wq': _pack_kxm(Wq.T.astype(f16)),             # [128,2,512]
        'wg2': _pack_kxm(Wg2.T.astype(f16)),           # [128,4,256]
        'wf1': _pack_kxm(Wf1.T.astype(f16)),           # [128,2,1024]
        'wf2': _pack_kxm(Wf2.T.astype(f16)),           # [128,8,256]
        'wb1': _pack_kxm(Wb1.T.astype(f16)),           # [128,2,64]
        'wb2': wb2_t,                                   # [64,9,64]
        'wb3': Wb3.T.astype(f16),                       # [64,256]
        'bt1': _pack_bias(t1),                          # [128,2] f32
        'bt2': _pack_bias(t2),
        'bpv': np.ascontiguousarray(bp.astype(f16).reshape(1, 512)),
        'bbf1': _pack_bias(bf1),                        # [128,8]
        'bbf2': _pack_bias(bf2),
        'btb1': np.ascontiguousarray(tb1[:, None].astype(np.float32)),  # [64,1]
        'btb2': np.ascontiguousarray(tb2[:, None].astype(np.float32)),
        'bsf': _pack_bias(sf),
        'btf2': _pack_bias(btf2),
    }


# --------------------------------------------------------------------------
# device kernel builder
# --------------------------------------------------------------------------
def _build_bass():
    import concourse.bass as bass
    import concourse.mybir as mybir
    from concourse import bacc
    from concourse.tile import TileContext
    from concourse.masks import make_identity

    dt = mybir.dt
    F16 = dt.float16
    F32 = dt.float32
    AF = mybir.ActivationFunctionType

    nc = bacc.Bacc()

    # ---- DRAM parameters ----
    xh_d = nc.declare_dram_parameter("xh", [IMGS_PER_CORE, C, N], F16, isOutput=False)
    wg1_d = nc.declare_dram_parameter("wg1", [128, 2, 256], F16, isOutput=False)
    wp_d = nc.declare_dram_parameter("wp", [128, 2, 512], F16, isOutput=False)
    wq_d = nc.declare_dram_parameter("wq", [128, 2, 512], F16, isOutput=False)
    wg2_d = nc.declare_dram_parameter("wg2", [128, 4, 256], F16, isOutput=False)
    wf1_d = nc.declare_dram_parameter("wf1", [128, 2, 1024], F16, isOutput=False)
    wf2_d = nc.declare_dram_parameter("wf2", [128, 8, 256], F16, isOutput=False)
    wb1_d = nc.declare_dram_parameter("wb1", [128, 2, 64], F16, isOutput=False)
    wb2_d = nc.declare_dram_parameter("wb2", [64, 9, 64], F16, isOutput=False)
    wb3_d = nc.declare_dram_parameter("wb3", [64, 256], F16, isOutput=False)
    bt1_d = nc.declare_dram_parameter("bt1", [128, 2], F32, isOutput=False)
    bt2_d = nc.declare_dram_parameter("bt2", [128, 2], F32, isOutput=False)
    bpv_d = nc.declare_dram_parameter("bpv", [1, 512], F16, isOutput=False)
    bbf1_d = nc.declare_dram_parameter("bbf1", [128, 8], F32, isOutput=False)
    bbf2_d = nc.declare_dram_parameter("bbf2", [128, 2], F32, isOutput=False)
    btb1_d = nc.declare_dram_parameter("btb1", [64, 1], F32, isOutput=False)
    btb2_d = nc.declare_dram_parameter("btb2", [64, 1], F32, isOutput=False)
    bsf_d = nc.declare_dram_parameter("bsf", [128, 2], F32, isOutput=False)
    btf2_d = nc.declare_dram_parameter("btf2", [128, 2], F32, isOutput=False)
    q_drams = [nc.dram_tensor(f"q_dram{i}", [N, 512], F16)
               for i in range(IMGS_PER_CORE)]
    out_d = nc.declare_dram_parameter("out", [IMGS_PER_CORE, C, N], F32, isOutput=True)

    with TileContext(nc) as tc:
        import contextlib
        ctx = contextlib.ExitStack()
        with ctx:
            consts = ctx.enter_context(tc.tile_pool(name="consts", bufs=1))
            # per-image pools: every tag is a dedicated buffer; PSUM pools are
            # per-image so slot rotation never chains one image behind the other
            pools = [ctx.enter_context(tc.tile_pool(name=f"im{i}", bufs=1))
                     for i in range(IMGS_PER_CORE)]
            psums = [ctx.enter_context(
                tc.tile_pool(name=f"psum{i}", bufs=3, space="PSUM"))
                for i in range(IMGS_PER_CORE)]
            psum64s = [ctx.enter_context(
                tc.tile_pool(name=f"psum64_{i}", bufs=1, space="PSUM"))
                for i in range(IMGS_PER_CORE)]
            pool_gath = ctx.enter_context(tc.tile_pool(name="gath", bufs=2))
            pool_sh = ctx.enter_context(tc.tile_pool(name="shared", bufs=1))

            # ---- constants / weights (loaded once) ----
            # alternate sync/scalar HWDGE queues so ~20 loads don't serialize
            # in front of the first matmul
            _ldq = [0]

            def load(name, shape, dtype, src, eng=None):
                t = consts.tile(shape, dtype, name=name)
                if eng is None:
                    eng = nc.sync if _ldq[0] % 2 == 0 else nc.scalar
                    _ldq[0] += 1
                eng.dma_start(out=t[:], in_=src[:])
                return t

            wg1 = load("wg1s", [128, 2, 256], F16, wg1_d, eng=nc.sync)
            bt1 = load("bt1s", [128, 2], F32, bt1_d, eng=nc.scalar)
            ones = consts.tile([128, 128], F16, name="ones")
            nc.gpsimd.memset(ones[:], 1.0)
            wp = load("wps", [128, 2, 512], F16, wp_d)
            wq = load("wqs", [128, 2, 512], F16, wq_d)
            wg2 = load("wg2s", [128, 4, 256], F16, wg2_d)
            wf1 = load("wf1s", [128, 2, 1024], F16, wf1_d)
            wf2 = load("wf2s", [128, 8, 256], F16, wf2_d)
            wb1 = load("wb1s", [128, 2, 64], F16, wb1_d)
            wb2 = load("wb2s", [64, 9, 64], F16, wb2_d)
            wb3 = load("wb3s", [64, 256], F16, wb3_d)
            bt2 = load("bt2s", [128, 2], F32, bt2_d)
            bpv = load("bpvs", [1, 512], F16, bpv_d)
            bbf1 = load("bbf1s", [128, 8], F32, bbf1_d)
            bbf2 = load("bbf2s", [128, 2], F32, bbf2_d)
            btb1 = load("btb1s", [64, 1], F32, btb1_d)
            btb2 = load("btb2s", [64, 1], F32, btb2_d)
            bsf = load("bsfs", [128, 2], F32, bsf_d)
            btf2 = load("btf2s", [128, 2], F32, btf2_d)

            ident = consts.tile([128, 128], F16, name="ident")
            make_identity(nc, ident[:])
            negid = consts.tile([128, 128], F16, name="negid")
            nc.scalar.activation(out=negid[:], in_=ident[:], func=AF.Copy,
                                 scale=NEG_BIG)
            # idbig[k, f] = 1 iff f == k + 384 (shifted identity for diag-kill)
            idbig = consts.tile([128, 1024], F16, name="idbig")
            nc.gpsimd.memset(idbig[:], 0.0)
            nc.gpsimd.affine_select(
                out=idbig[:], in_=idbig[:],
                compare_op=mybir.AluOpType.not_equal, fill=1.0,
                base=384, pattern=[[-1, 1024]], channel_multiplier=1)
            epsb = consts.tile([128, 1], F32, name="epsb")
            nc.gpsimd.memset(epsb[:], 1e-12)
            # dummy gather: loads the gather ucode into Q7 IRAM now instead of
            # in front of image 0's first real gather
            zidx = consts.tile([128, 8], dt.int16, name="zidx")
            nc.gpsimd.memset(zidx[:], 0)
            scrg = consts.tile([128, 1, 512], F16, name="scrg")
            nc.gpsimd.dma_gather(
                out_ap=scrg[:], in_ap=q_drams[0][:], idxs_ap=zidx[:],
                num_idxs=128, num_idxs_reg=128, elem_size=512,
                transpose=False, single_packet=False)

            # per-image tile state
            st = [{} for _ in range(IMGS_PER_CORE)]

            def T(img, tag, shape, dtype):
                t = pools[img].tile(shape, dtype, name=f"{tag}_{img}", tag=tag)
                st[img][tag] = t
                return t

            def phase_pre(img):
                """load x, g1 conv, feature norms."""
                xh = T(img, "xh", [128, 2, N], F16)
                for t in range(2):
                    nc.sync.dma_start(out=xh[:, t, :],
                                      in_=xh_d[img, t * 128:(t + 1) * 128, :])
                featT = T(img, "feat", [128, 2, N], F16)
                for to in range(2):
                    pss = [psums[img].tile([128, 512], F32,
                                           name=f"ps_g1_{img}_{to}_{nb}", tag="ps")
                           for nb in range(2)]
                    for kt in range(2):
                        for nb in range(2):
                            nc.tensor.matmul(
                                pss[nb][:], lhsT=wg1[:, kt, to * 128:(to + 1) * 128],
                                rhs=xh[:, kt, nb * 512:(nb + 1) * 512],
                                start=(kt == 0), stop=(kt == 1))
                    for nb in range(2):
                        nc.scalar.activation(
                            out=featT[:, to, nb * 512:(nb + 1) * 512], in_=pss[nb][:],
                            func=AF.Identity, bias=bt1[:, to:to + 1])
                fsq = T(img, "fx1", [128, 2, N], F16)
                for t in range(2):
                    nc.vector.tensor_mul(fsq[:, t, :], featT[:, t, :], featT[:, t, :])
                n2b = T(img, "n2b", [128, N], F32)
                for nb in range(2):
                    psn = psums[img].tile([128, 512], F32,
                                          name=f"ps_n2_{img}_{nb}", tag="ps")
                    for kt in range(2):
                        nc.tensor.matmul(
                            psn[:], lhsT=ones[:],
                            rhs=fsq[:, kt, nb * 512:(nb + 1) * 512],
                            start=(kt == 0), stop=(kt == 1))
                    nc.scalar.activation(out=n2b[:, nb * 512:(nb + 1) * 512],
                                         in_=psn[:], func=AF.Identity,
                                         bias=epsb[:, 0:1])
                rn2 = T(img, "fx1", [128, N], F32)  # reuses fsq slot (fsq dead)
                nc.vector.reciprocal_approx_fast(out=rn2[:], in_=n2b[:])
                invnb = T(img, "invnb", [128, N], F16)
                nc.scalar.activation(out=invnb[:], in_=rn2[:], func=AF.Sqrt)
                xnT = T(img, "n2b", [128, 2, N], F16)  # reuses n2b slot
                for t in range(2):
                    nc.vector.tensor_mul(xnT[:, t, :], featT[:, t, :], invnb[:])

            def phase_sel(img, Is):
                """sim I-blocks + top-8 selection for I in Is."""
                featT = st[img]["feat"]
                xnT = st[img]["n2b"]
                if 0 in Is:
                    st[img]["ixt"] = T(img, "ix", [128, 4, 8, 2], dt.uint16)
                ixbuf = st[img]["ixt"]
                for I in Is:
                    simblk = pools[img].tile([128, N], F16, name=f"sim{img}_{I}",
                                             tag="sim", bufs=2)
                    pss = [psums[img].tile([128, 512], F32,
                                           name=f"ps_sim_{img}_{I}_{cb}", tag="ps")
                           for cb in range(2)]
                    for kt in range(2):
                        for cb in range(2):
                            has_diag = (cb == I // 4)
                            nc.tensor.matmul(
                                pss[cb][:], lhsT=featT[:, kt, I * 128:(I + 1) * 128],
                                rhs=xnT[:, kt, cb * 512:(cb + 1) * 512],
                                start=(kt == 0),
                                stop=(kt == 1 and not has_diag))
                    for cb in range(2):
                        if cb == I // 4:
                            off = I * 128 - cb * 512
                            nc.tensor.matmul(pss[cb][:], lhsT=negid[:],
                                             rhs=idbig[:, 384 - off:896 - off],
                                             start=False, stop=True)
                        nc.scalar.activation(
                            out=simblk[:, cb * 512:(cb + 1) * 512], in_=pss[cb][:],
                            func=AF.Copy)
                    mx = pools[img].tile([128, 8], F16, name=f"mx{img}_{I}",
                                         tag="mx", bufs=2)
                    nc.vector.max(out=mx[:], in_=simblk[:])
                    nc.vector.max_index(out=ixbuf[:, I // 2, :, I % 2],
                                        in_max=mx[:], in_values=simblk[:])

            def phase_wrap(img):
                """wrapped idx buffer [128, 512] i16.
                col = 128*s + 16*k + 8*i + g ; idx number within s-block
                = 256*k + 128*i + 16*g + p16 -> node 256*s + 128*i + 16*g + p16
                """
                wrapped = T(img, "wrap", [128, 512], dt.int16)
                wtmp = T(img, "wtmp", [16, 8, 64], dt.int16)
                ixi = st[img]["ix"][:].bitcast(dt.int16)
                ixf = ixi.rearrange("p s k i -> p (s k i)")
                for g in range(8):
                    nc.sync.dma_start(out=wtmp[:, g, :],
                                      in_=ixf[16 * g:16 * (g + 1), :])
                # per-partition (g,s,k,i) -> (s,k,i,g) permute on DVE
                nc.vector.tensor_copy(
                    wrapped[0:16, :].rearrange(
                        "p (s k i g) -> p s k i g", s=4, k=8, i=2, g=8),
                    wtmp[:].rearrange("p g (s k i) -> p s k i g", s=4, k=8, i=2))
                nc.sync.dma_start(out=wrapped[16:32, :], in_=wrapped[0:16, :])
                nc.sync.dma_start(out=wrapped[32:64, :], in_=wrapped[0:32, :])
                nc.sync.dma_start(out=wrapped[64:128, :], in_=wrapped[0:64, :])

            def phase_qp(img):
                """q (n-major, to DRAM for gather) and p (n-major, +bp bias)."""
                featT = st[img]["feat"]
                q_sb = T(img, "q", [128, 8, 512], F16)
                q_dram = q_drams[img]
                for nt in range(8):
                    ps = psums[img].tile([128, 512], F32,
                                         name=f"ps_q_{img}_{nt}", tag="ps")
                    for kt in range(2):
                        nc.tensor.matmul(
                            ps[:], lhsT=featT[:, kt, nt * 128:(nt + 1) * 128],
                            rhs=wq[:, kt, :], start=(kt == 0), stop=(kt == 1))
                    nc.scalar.activation(out=q_sb[:, nt, :], in_=ps[:], func=AF.Copy)
                    nc.sync.dma_start(out=q_dram[nt * 128:(nt + 1) * 128, :],
                                      in_=q_sb[:, nt, :])
                p_nm = T(img, "pT", [128, 8, 512], F16)
                st[img]["p_nm"] = p_nm
                for nt in range(8):
                    ps = psums[img].tile([128, 512], F32,
                                         name=f"ps_p_{img}_{nt}", tag="ps")
                    for kt in range(2):
                        nc.tensor.matmul(
                            ps[:], lhsT=featT[:, kt, nt * 128:(nt + 1) * 128],
                            rhs=wp[:, kt, :], start=(kt == 0), stop=False)
                    # per-channel EdgeConv bias via K=1 broadcast matmul
                    nc.tensor.matmul(ps[:], lhsT=ones[0:1, :], rhs=bpv[0:1, :],
                                     start=False, stop=True)
                    nc.scalar.activation(out=p_nm[:, nt, :], in_=ps[:], func=AF.Copy)

            def phase_gather_dma(img, s):
                """issue the neighbor gather for s-block (n-major output)."""
                if s == 0:
                    st[img]["maxqt"] = T(img, "maxq", [128, 8, 512], F16)
                wrapped = st[img]["wrap"]
                go = pool_gath.tile([128, 16, 512], F16, name=f"go{img}_{s}",
                                    tag="go")
                st[img][f"go{s}"] = go
                nc.gpsimd.dma_gather(
                    out_ap=go[:], in_ap=q_drams[img][:],
                    idxs_ap=wrapped[:, 128 * s:128 * (s + 1)],
                    num_idxs=2048, num_idxs_reg=2048, elem_size=512,
                    transpose=False, single_packet=False)

            def phase_fold(img, s):
                """8-way max fold + self term for s-block."""
                maxq = st[img]["maxqt"]
                q_sb = st[img]["q"]
                go = st[img][f"go{s}"]
                gv = go[:].rearrange("p (k h) c -> p k h c", k=8)
                nc.vector.tensor_max(gv[:, 4:8, :, :], gv[:, 0:4, :, :],
                                     gv[:, 4:8, :, :])
                nc.vector.tensor_max(gv[:, 6:8, :, :], gv[:, 4:6, :, :],
                                     gv[:, 6:8, :, :])
                nc.vector.tensor_max(gv[:, 7, :, :], gv[:, 6, :, :],
                                     gv[:, 7, :, :])
                nc.vector.tensor_max(maxq[:, 2 * s:2 * s + 2, :],
                                     gv[:, 7, :, :], q_sb[:, 2 * s:2 * s + 2, :])

            def phase_edge(img, nbs=(0, 1)):
                """e = p + maxq in n-major, then PE-transpose to ch-major
                eT [128, 4, N]; relu rides the transpose evacuation."""
                p_nm = st[img]["p_nm"]
                maxq = st[img]["maxqt"]
                if 0 in nbs:
                    st[img]["e_nm"] = T(img, "n2b", [128, 8, 512], F16)
                    st[img]["eTt"] = T(img, "fx1", [128, 4, N], F16)
                e_nm = st[img]["e_nm"]
                eT = st[img]["eTt"]
                for nb in nbs:
                    nc.vector.tensor_add(
                        e_nm[:, 4 * nb:4 * (nb + 1), :].rearrange(
                            "p a c -> p (a c)"),
                        p_nm[:, 4 * nb:4 * (nb + 1), :].rearrange(
                            "p a c -> p (a c)"),
                        maxq[:, 4 * nb:4 * (nb + 1), :].rearrange(
                            "p a c -> p (a c)"))
                    for a in range(4):
                        ps = psums[img].tile([128, 512], F16,
                                             name=f"ps_tr_{img}_{a}_{nb}", tag="ps")
                        for j in range(4):
                            nc.tensor.transpose(
                                out=ps[:, j * 128:(j + 1) * 128],
                                in_=e_nm[:, 4 * nb + j, a * 128:(a + 1) * 128],
                                identity=ident[:])
                        nc.scalar.activation(
                            out=eT[:, a, nb * 512:(nb + 1) * 512], in_=ps[:],
                            func=AF.Relu)

            def phase_g2(img, dve_evac=False, nbs=(0, 1)):
                """g2 conv + residual (ident@xh in PSUM) -> hc f16."""
                eT = st[img]["eTt"]
                xh = st[img]["xh"]
                if 0 in nbs:
                    st[img]["hct"] = T(img, "feat", [128, 2, N], F16)
                hc = st[img]["hct"]
                for to in range(2):
                    for nb in nbs:
                        ps = psums[img].tile([128, 512], F32,
                                             name=f"ps_g2_{img}_{to}_{nb}", tag="ps")
                        for kt in range(4):
                            nc.tensor.matmul(
                                ps[:], lhsT=wg2[:, kt, to * 128:(to + 1) * 128],
                                rhs=eT[:, kt, nb * 512:(nb + 1) * 512],
                                start=(kt == 0), stop=False)
                        nc.tensor.matmul(
                            ps[:], lhsT=ident[:],
                            rhs=xh[:, to, nb * 512:(nb + 1) * 512],
                            start=False, stop=True)
                        if dve_evac and nb == 1:
                            nc.vector.tensor_scalar_add(
                                hc[:, to, nb * 512:(nb + 1) * 512], ps[:],
                                bt2[:, to:to + 1])
                        else:
                            nc.scalar.activation(
                                out=hc[:, to, nb * 512:(nb + 1) * 512],
                                in_=ps[:],
                                func=AF.Identity, bias=bt2[:, to:to + 1])

            def phase_f1(img, dve_evac=False, nbs=(0, 1)):
                hc = st[img]["hct"]
                if 0 in nbs:
                    # shared across images: img0's f1o is dead (f2(0) done)
                    # before img1's f1 starts
                    st[img]["f1ot"] = pool_sh.tile(
                        [128, 8, N], F16, name=f"f1o_{img}", tag="f1o")
                f1o = st[img]["f1ot"]
                for to in range(8):
                    for nb in nbs:
                        ps = psums[img].tile([128, 512], F32,
                                             name=f"ps_f1_{img}_{to}_{nb}", tag="ps")
                        for kt in range(2):
                            nc.tensor.matmul(
                                ps[:], lhsT=wf1[:, kt, to * 128:(to + 1) * 128],
                                rhs=hc[:, kt, nb * 512:(nb + 1) * 512],
                                start=(kt == 0), stop=(kt == 1))
                        dst = f1o[:, to, nb * 512:(nb + 1) * 512]
                        if dve_evac and to % 2 == 1:
                            nc.vector.tensor_scalar(
                                out=dst, in0=ps[:],
                                scalar1=bbf1[:, to:to + 1], scalar2=0.0,
                                op0=mybir.AluOpType.add,
                                op1=mybir.AluOpType.max)
                        else:
                            nc.scalar.activation(
                                out=dst, in_=ps[:],
                                func=AF.Relu, bias=bbf1[:, to:to + 1])

            def phase_f2(img, dve_evac=False, nbs=(0, 1)):
                f1o = st[img]["f1ot"]
                hc = st[img]["hct"]
                if 0 in nbs:
                    st[img]["h2ct"] = T(img, "h2c", [128, 2, N], F16)
                h2c = st[img]["h2ct"]
                for to in range(2):
                    for nb in nbs:
                        ps = psums[img].tile([128, 512], F32,
                                             name=f"ps_f2_{img}_{to}_{nb}", tag="ps")
                        for kt in range(8):
                            nc.tensor.matmul(
                                ps[:], lhsT=wf2[:, kt, to * 128:(to + 1) * 128],
                                rhs=f1o[:, kt, nb * 512:(nb + 1) * 512],
                                start=(kt == 0), stop=False)
                        nc.tensor.matmul(
                            ps[:], lhsT=ident[:],
                            rhs=hc[:, to, nb * 512:(nb + 1) * 512],
                            start=False, stop=True)
                        if dve_evac and nb == 1:
                            nc.vector.tensor_scalar_add(
                                h2c[:, to, nb * 512:(nb + 1) * 512], ps[:],
                                bbf2[:, to:to + 1])
                        else:
                            nc.scalar.activation(
                                out=h2c[:, to, nb * 512:(nb + 1) * 512],
                                in_=ps[:],
                                func=AF.Identity, bias=bbf2[:, to:to + 1])

            def phase_bneck(img):
                h2c = st[img]["h2ct"]
                pad = T(img, "pad", [64, 34 * 34], F16)
                nc.vector.memset(pad[:], 0.0)
                pad3 = pad[:].rearrange("p (r c) -> p r c", r=34)
                for nb in range(2):
                    ps = psum64s[img].tile([64, 512], F32,
                                           name=f"ps_b1_{img}_{nb}", tag="ps64")
                    for kt in range(2):
                        nc.tensor.matmul(
                            ps[:], lhsT=wb1[:, kt, :],
                            rhs=h2c[:, kt, nb * 512:(nb + 1) * 512],
                            start=(kt == 0), stop=(kt == 1))
                    # evacuate straight into the zero-padded conv input
                    nc.scalar.activation(
                        out=pad3[:, 1 + 16 * nb:17 + 16 * nb, 1:33],
                        in_=ps[:].rearrange("p (r c) -> p r c", r=16),
                        func=AF.Relu, bias=btb1[:, 0:1])
                b2o = T(img, "b2o", [64, N], F16)
                for nb in range(2):
                    ps = psum64s[img].tile([64, 512], F32,
                                           name=f"ps_b2_{img}_{nb}", tag="ps64")
                    for tap in range(9):
                        dy, dx = tap // 3, tap % 3
                        rhs = pad3[:, 16 * nb + dy:16 * nb + dy + 16, dx:dx + 32]
                        nc.tensor.matmul(ps[:], lhsT=wb2[:, tap, :], rhs=rhs,
                                         start=(tap == 0), stop=(tap == 8))
                    nc.scalar.activation(out=b2o[:, nb * 512:(nb + 1) * 512],
                                         in_=ps[:], func=AF.Relu, bias=btb2[:, 0:1])

            def phase_out(img, dve_evac=False):
                """b3 + h2 + x residuals in PSUM; final BN in the evacuation."""
                h2c = st[img]["h2ct"]
                xh = st[img]["xh"]
                b2o = st[img]["b2o"]
                out32 = T(img, "maxq", [128, 2, N], F32)  # reuses maxq slot
                for to in range(2):
                    pss = [psums[img].tile([128, 512], F32,
                                           name=f"ps_b3_{img}_{to}_{nb}", tag="ps")
                           for nb in range(2)]
                    for nb in range(2):
                        sl = slice(nb * 512, (nb + 1) * 512)
                        nc.tensor.matmul(
                            pss[nb][:], lhsT=wb3[:, to * 128:(to + 1) * 128],
                            rhs=b2o[:, sl], start=True, stop=False)
                        nc.tensor.matmul(
                            pss[nb][:], lhsT=ident[:], rhs=h2c[:, to, sl],
                            start=False, stop=False)
                        nc.tensor.matmul(
                            pss[nb][:], lhsT=ident[:], rhs=xh[:, to, sl],
                            start=False, stop=True)
                        if dve_evac and nb == 1:
                            nc.vector.tensor_scalar(
                                out=out32[:, to, sl], in0=pss[nb][:],
                                scalar1=bsf[:, to:to + 1],
                                scalar2=btf2[:, to:to + 1],
                                op0=mybir.AluOpType.mult,
                                op1=mybir.AluOpType.add)
                        else:
                            nc.scalar.activation(
                                out=out32[:, to, sl], in_=pss[nb][:],
                                func=AF.Identity, scale=bsf[:, to:to + 1],
                                bias=btf2[:, to:to + 1])
                    for nb in range(2):
                        nc.sync.dma_start(
                            out=out_d[img, to * 128:(to + 1) * 128,
                                      nb * 512:(nb + 1) * 512],
                            in_=out32[:, to, nb * 512:(nb + 1) * 512])

            # ---- two-image software pipeline (emission order = priority;
            # engine queues execute in order, so image 1's DVE/PE work must be
            # emitted BEFORE image 0's folds or it stalls behind them) ----
            phase_pre(0)
            phase_sel(0, range(8))
            phase_qp(0)
            phase_wrap(0)
            phase_gather_dma(0, 0)
            phase_pre(1)
            phase_fold(0, 0)
            phase_sel(1, range(0, 4))
            phase_gather_dma(0, 1)
            phase_fold(0, 1)
            phase_sel(1, range(4, 8))
            phase_gather_dma(0, 2)
            phase_fold(0, 2)
            phase_qp(1)
            phase_wrap(1)
            phase_gather_dma(0, 3)
            phase_fold(0, 3)
            phase_edge(0)
            phase_g2(0)
            phase_gather_dma(1, 0)
            phase_fold(1, 0)
            phase_f1(0)
            phase_gather_dma(1, 1)
            phase_fold(1, 1)
            phase_f2(0)
            phase_gather_dma(1, 2)
            phase_fold(1, 2)
            phase_edge(1, nbs=(0,))
            phase_g2(1, nbs=(0,))
            phase_bneck(0)
            phase_f1(1, dve_evac=True, nbs=(0,))
            phase_gather_dma(1, 3)
            phase_fold(1, 3)
            phase_f2(1, dve_evac=True, nbs=(0,))
            phase_out(0)
            phase_edge(1, nbs=(1,))
            phase_g2(1, dve_evac=True, nbs=(1,))
            phase_f1(1, dve_evac=True, nbs=(1,))
            phase_f2(1, dve_evac=True, nbs=(1,))
            phase_bneck(1)
            phase_out(1, dve_evac=True)

    nc.finalize()
    return nc


# --------------------------------------------------------------------------
# entry point
# --------------------------------------------------------------------------
def kernel(**inputs):
    inp = {k: np.asarray(v) for k, v in inputs.items()}
    w = _prep_weights(inp)

    if 'nc' not in _cache:
        _cache['nc'] = _build_bass()
    nc = _cache['nc']

    xh = inp['x'].astype(np.float16).reshape(B, C, N)
    in_maps = []
    for c in range(N_CORES):
        m = {'xh': np.ascontiguousarray(xh[c * 2:(c + 1) * 2])}
        m.update(w)
        in_maps.append(m)

    from concourse.bass_utils import run_bass_kernel_spmd
    trace = bool(os.environ.get("KBENCH_TRACE"))
    res = run_bass_kernel_spmd(nc, in_maps, core_ids=list(range(N_CORES)),
                               trace=trace)
    _cache['exec_time_ns'] = res.exec_time_ns
    _cache['results'] = res
    out = np.zeros((B, C, N), np.float32)
    for c in range(N_CORES):
        out[c * 2:(c + 1) * 2] = res.results[c]['out']
    return out.reshape(B, C, H, W)
